# revision 1
# baseline (speedup 1.0000x reference)
"""Trainium2 Bass kernel for nn_BasicTransformerBlock_35304631173827.

Sharding: 8 cores = 4 samples x 2 sequence halves. Each core computes its
1024-token half of one sample fully locally (self-attention K/V recomputed
over the full 2048-token sample -> zero collectives). bf16 matmuls with
fp32 PSUM accumulation; LayerNorm stats, softmax and residuals in fp32.
Large intermediates (h1T, kT, x1, x2, y-accumulator) bounce through DRAM
to fit SBUF.
"""

import numpy as np
import ml_dtypes

BF16 = ml_dtypes.bfloat16

B, N, D = 4, 2048, 1024
J, CD = 256, 768
H, DH = 16, 64
INNER = 1024
FF = 4096
P = 128
KT = D // P            # 8
CKT = CD // P          # 6
TT_FULL = N // P       # 16
N_OWN = N // 2
TT_OWN = N_OWN // P    # 8
EPS = 1e-5

_CACHE = {}


def _build_program():
    import concourse.tile as tile
    from concourse import mybir, bacc
    from concourse.masks import make_identity
    from contextlib import ExitStack

    f32 = mybir.dt.float32
    bf16 = mybir.dt.bfloat16
    AF = mybir.ActivationFunctionType
    ALU = mybir.AluOpType

    nc = bacc.Bacc(None, target_bir_lowering=False)

    xf_d = nc.dram_tensor("xf", [TT_FULL, P, D], f32, kind="ExternalInput")
    tT_d = nc.dram_tensor("tT", [P, KT], bf16, kind="ExternalInput")
    nw_d = nc.dram_tensor("nw", [P, KT, 6 * D], bf16, kind="ExternalInput")
    nbc_d = nc.dram_tensor("nbc", [P, 48], f32, kind="ExternalInput")
    wq1_d = nc.dram_tensor("wq1", [P, KT, INNER], bf16, kind="ExternalInput")
    wk1_d = nc.dram_tensor("wk1", [P, KT, INNER], bf16, kind="ExternalInput")
    wv1_d = nc.dram_tensor("wv1", [P, KT, INNER], bf16, kind="ExternalInput")
    wo1_d = nc.dram_tensor("wo1", [P, KT, D], bf16, kind="ExternalInput")
    wq2_d = nc.dram_tensor("wq2", [P, KT, INNER], bf16, kind="ExternalInput")
    wk2_d = nc.dram_tensor("wk2", [P, CKT, INNER], bf16, kind="ExternalInput")
    wv2_d = nc.dram_tensor("wv2", [P, CKT, INNER], bf16, kind="ExternalInput")
    wo2_d = nc.dram_tensor("wo2", [P, KT, D], bf16, kind="ExternalInput")
    ctxT_d = nc.dram_tensor("ctxT", [P, CKT, J], bf16, kind="ExternalInput")
    bias3_d = nc.dram_tensor("bias3", [P, 3, D], f32, kind="ExternalInput")
    fb1_d = nc.dram_tensor("fb1c", [P, 64], f32, kind="ExternalInput")
    wf1_d = nc.dram_tensor("wf1", [P, KT, 2 * FF], bf16, kind="ExternalInput")
    wf2_d = nc.dram_tensor("wf2", [P, FF // P, D], bf16, kind="ExternalInput")
    y_d = nc.dram_tensor("y", [TT_OWN, P, D], f32, kind="ExternalOutput")

    with tile.TileContext(nc) as tc, ExitStack() as es:
        konst = es.enter_context(tc.tile_pool(name="konst", bufs=1))
        xpool = es.enter_context(tc.tile_pool(name="xpool", bufs=3))
        stats = es.enter_context(tc.tile_pool(name="stats", bufs=2))
        wres = es.enter_context(tc.tile_pool(name="wres", bufs=2))
        wsm = es.enter_context(tc.tile_pool(name="wsm", bufs=3))
        wmed = es.enter_context(tc.tile_pool(name="wmed", bufs=2))
        evict = es.enter_context(tc.tile_pool(name="evict", bufs=2))
        stg = es.enter_context(tc.tile_pool(name="stg", bufs=2))
        big = es.enter_context(tc.tile_pool(name="big", bufs=1))
        kthp = es.enter_context(tc.tile_pool(name="kthp", bufs=2))
        expp = es.enter_context(tc.tile_pool(name="expp", bufs=2))
        smk = es.enter_context(tc.tile_pool(name="smk", bufs=1))
        dramp = es.enter_context(tc.tile_pool(name="dramp", bufs=1, space="DRAM"))
        ps_a = es.enter_context(tc.tile_pool(name="ps_a", bufs=2, space="PSUM"))
        ps_sc = es.enter_context(tc.tile_pool(name="ps_sc", bufs=2, space="PSUM"))
        ps_av = es.enter_context(tc.tile_pool(name="ps_av", bufs=2, space="PSUM"))
        ps_tr = ps_av

        # ---------------- constants ----------------
        ident = konst.tile([P, P], bf16)
        make_identity(nc, ident)
        ones64 = konst.tile([1, 64], bf16)
        nc.vector.memset(ones64[:], 1.0)
        eps_t = konst.tile([P, 1], f32)
        nc.vector.memset(eps_t[:], EPS)
        tT_sb = konst.tile([P, KT], bf16)
        nc.sync.dma_start(tT_sb[:], tT_d[:])
        nbc_sb = konst.tile([P, 48], f32)
        nc.sync.dma_start(nbc_sb[:], nbc_d[:])
        fb1_sb = konst.tile([P, 64], f32)
        nc.sync.dma_start(fb1_sb[:], fb1_d[:])
        ctxT_sb = konst.tile([P, CKT, J], bf16)
        nc.sync.dma_start(ctxT_sb[:], ctxT_d[:])
        cols = konst.tile([P, 48], f32)

        # DRAM scratch (ExternalOutput so they double as debug dumps)
        h1T_dram = nc.dram_tensor("dbg_h1T", [P, KT, N], bf16, kind="ExternalOutput")
        kT_dram = nc.dram_tensor("dbg_kT", [KT, P, N], bf16, kind="ExternalOutput")
        x1_dram = nc.dram_tensor("dbg_x1", [TT_OWN, P, D], f32, kind="ExternalOutput")
        x2_dram = nc.dram_tensor("dbg_x2", [TT_OWN, P, D], f32, kind="ExternalOutput")
        dbg_cols = nc.dram_tensor("dbg_cols", [P, 48], f32, kind="ExternalOutput")
        dbg_q = nc.dram_tensor("dbg_q", [P, KT, N_OWN], bf16, kind="ExternalOutput")
        dbg_a1 = nc.dram_tensor("dbg_a1", [P, KT, N_OWN], bf16, kind="ExternalOutput")

        # ---------------- Phase 0: AdaLN embeddings (transposed: M=128,N=1) ----
        for c in range(48):
            nwt = wsm.tile([P, KT, P], bf16, tag="wstream")
            nc.sync.dma_start(nwt[:], nw_d[:, :, c * P:(c + 1) * P])
            ps = ps_a.tile([P, 512], f32, tag="psa")
            for kt in range(KT):
                nc.tensor.matmul(ps[:, 0:1], nwt[:, kt, :], tT_sb[:, kt:kt + 1],
                                 start=(kt == 0), stop=(kt == KT - 1))
            nc.vector.tensor_copy(cols[:, c:c + 1], ps[:, 0:1])
        nc.vector.tensor_add(cols[:], cols[:], nbc_sb[:])
        for n3 in range(3):
            nc.vector.tensor_scalar_add(cols[:, n3 * 16:n3 * 16 + 8],
                                        cols[:, n3 * 16:n3 * 16 + 8], 1.0)

        nc.sync.dma_start(dbg_cols[:], cols[:])

        def layernorm_tile(x_tile, tt, n3, dst_sb=None, dst_dram=None):
            """LayerNorm + AdaLN affine on (P, D) tile -> transposed chunks."""
            bst = stats.tile([P, 2, 6], f32, tag="bnst")
            for g in range(2):
                nc.vector.bn_stats(bst[:, g, :], x_tile[:, g * 512:(g + 1) * 512])
            mv = stats.tile([P, 4], f32, tag="mv")
            nc.vector.bn_aggr(mv[:, 0:2], bst[:])
            nc.scalar.activation(mv[:, 2:3], mv[:, 1:2], AF.Sqrt, bias=eps_t[:])
            nc.vector.reciprocal(mv[:, 2:3], mv[:, 2:3])
            nc.vector.tensor_tensor(mv[:, 3:4], mv[:, 0:1], mv[:, 2:3], ALU.mult)
            nc.vector.tensor_scalar_mul(mv[:, 3:4], mv[:, 3:4], -1.0)
            xn = evict.tile([P, D], bf16, tag="xn")
            nc.scalar.activation(xn[:], x_tile[:], AF.Identity,
                                 bias=mv[:, 3:4], scale=mv[:, 2:3])
            if dst_dram is not None:
                stage = stg.tile([P, KT, P], bf16, tag="stage", name="stage")
            else:
                stage = None
            for c in range(KT):
                pt = ps_tr.tile([P, P], bf16, tag="psav")
                nc.tensor.transpose(pt[:], xn[:, c * P:(c + 1) * P], ident[:])
                out_ap = (stage[:, c, :] if dst_dram is not None
                          else dst_sb[:, c, tt * P:(tt + 1) * P])
                nc.vector.tensor_scalar(
                    out_ap, pt[:],
                    cols[:, n3 * 16 + c:n3 * 16 + c + 1],
                    cols[:, n3 * 16 + 8 + c:n3 * 16 + 8 + c + 1],
                    ALU.mult, ALU.add)
            if dst_dram is not None:
                nc.sync.dma_start(dst_dram[:, :, tt * P:(tt + 1) * P], stage[:])

        # ---------------- Phase 1: LN1 (full sample) -> h1T_dram ----------------
        for tt in range(TT_FULL):
            xt = xpool.tile([P, D], f32, tag="x")
            nc.sync.dma_start(xt[:], xf_d[tt])
            layernorm_tile(xt, tt, 0, dst_dram=h1T_dram)

        # ---------------- Phase 2: QKV projections ----------------
        qT = big.tile([P, KT, N_OWN], bf16, tag="qT")
        SC = DH ** -0.5

        def qk_proj(w_dram, n_tok, out_sb, out_dram, scale):
            w_sb = wres.tile([P, KT, INNER], bf16, tag="wbig")
            nc.sync.dma_start(w_sb[:], w_dram[:])
            for qc in range(n_tok // 256):
                hch = wmed.tile([P, KT, 256], bf16, tag="med4")
                nc.sync.dma_start(hch[:], h1T_dram[:, :, qc * 256:(qc + 1) * 256])
                for m in range(KT):
                    ps = ps_a.tile([P, 512], f32, tag="psa")
                    for kt in range(KT):
                        nc.tensor.matmul(ps[:, 0:256],
                                         w_sb[:, kt, m * P:(m + 1) * P],
                                         hch[:, kt, :],
                                         start=(kt == 0), stop=(kt == KT - 1))
                    if out_sb is not None:
                        nc.vector.tensor_scalar_mul(
                            out_sb[:, m, qc * 256:(qc + 1) * 256], ps[:, 0:256], scale)
                    else:
                        kst = stg.tile([P, 256], bf16, tag="kstage")
                        nc.vector.tensor_copy(kst[:], ps[:, 0:256])
                        nc.sync.dma_start(
                            out_dram[m, :, qc * 256:(qc + 1) * 256], kst[:])

        qk_proj(wq1_d, N_OWN, qT, None, SC)
        nc.sync.dma_start(dbg_q[:], qT[:])
        qk_proj(wk1_d, N, None, kT_dram, None)

        v_sb = big.tile([P, TT_FULL, H, DH + 1], bf16, tag="v33")
        nc.vector.memset(v_sb[:], 1.0)
        w_sb = wres.tile([P, KT, INNER], bf16, tag="wbig")
        nc.sync.dma_start(w_sb[:], wv1_d[:])
        for tt in range(TT_FULL):
            hch = wmed.tile([P, KT, P], bf16, tag="med4")
            nc.sync.dma_start(hch[:], h1T_dram[:, :, tt * P:(tt + 1) * P])
            for nc2 in range(2):
                ps = ps_a.tile([P, 512], f32, tag="psa")
                for kt in range(KT):
                    nc.tensor.matmul(ps[:], hch[:, kt, :],
                                     w_sb[:, kt, nc2 * 512:(nc2 + 1) * 512],
                                     start=(kt == 0), stop=(kt == KT - 1))
                nc.vector.tensor_copy(
                    v_sb[:, tt, nc2 * 8:(nc2 + 1) * 8, 0:DH],
                    ps[:].rearrange("p (hh r) -> p hh r", r=DH))

        # ---------------- attention (shared for self / cross) ----------------
        def attention(get_k, v_t, qT_t, n_keys_tt, out_T):
            for h in range(H):
                hp = (h % 2) * 64
                m2 = h // 2
                kap = get_k(h)  # (P, n_keys) tile; head at partitions hp:hp+64
                for qc in range(2):
                    ex = expp.tile([P, n_keys_tt, 512], bf16, tag="expT")
                    for kt2 in range(max(1, n_keys_tt // 2)):
                        ps_s = ps_sc.tile([P, 1024], f32, tag="pssc")
                        for u in range(min(2, n_keys_tt)):
                            kt = kt2 * 2 + u
                            nc.tensor.matmul(
                                ps_s[:, u * 512:(u + 1) * 512],
                                kap[hp:hp + 64, kt * P:(kt + 1) * P],
                                qT_t[hp:hp + 64, m2, qc * 512:(qc + 1) * 512],
                                start=True, stop=True)
                        nkk = min(2, n_keys_tt)
                        nc.scalar.activation(
                            ex[:, kt2 * 2:kt2 * 2 + nkk, :].rearrange("p a b -> p (a b)"),
                            ps_s[:, 0:nkk * 512], AF.Exp)
                    pavt = ps_av.tile([P, 512], f32, tag="psav")
                    for kt in range(n_keys_tt):
                        fl = dict(start=(kt == 0), stop=(kt == n_keys_tt - 1))
                        if hp == 0:
                            nc.tensor.matmul(pavt[0:DH + 1], v_t[:, kt, h, :],
                                             ex[:, kt, :], **fl)
                        else:
                            nc.tensor.matmul(pavt[64:P], v_t[:, kt, h, 0:DH],
                                             ex[:, kt, :], **fl)
                            nc.tensor.matmul(pavt[0:1], v_t[:, kt, h, DH:DH + 1],
                                             ex[:, kt, :], **fl)
                    sumrow = pavt[DH:DH + 1] if hp == 0 else pavt[0:1]
                    rec = stats.tile([1, 512], bf16, tag="rec")
                    with nc.allow_low_precision(reason="softmax denom bcast"):
                        nc.vector.reciprocal(rec[:], sumrow[:])
                    pbc = ps_a.tile([P, 512], f32, tag="psa")
                    nc.tensor.matmul(pbc[hp:hp + 64, :], ones64[:], rec[:],
                                     start=True, stop=True)
                    bcs = stats.tile([P, 512], f32, tag="bcs")
                    nc.vector.tensor_copy(bcs[hp:hp + 64, :], pbc[hp:hp + 64, :])
                    nc.vector.tensor_tensor(
                        out_T[hp:hp + 64, m2, qc * 512:(qc + 1) * 512],
                        pavt[hp:hp + 64, :], bcs[hp:hp + 64, :], ALU.mult)

        # ---------------- Phase 3: self-attention ----------------
        attn1T = big.tile([P, KT, N_OWN], bf16, tag="attnT")
        _kcache = {}

        def get_k_self(h):
            m2 = h // 2
            if m2 not in _kcache:
                kth = kthp.tile([P, N], bf16, tag="kTh", name="kth")
                nc.sync.dma_start(kth[:], kT_dram[m2])
                _kcache.clear()
                _kcache[m2] = kth
            return _kcache[m2]

        attention(get_k_self, v_sb, qT, TT_FULL, attn1T)
        nc.sync.dma_start(dbg_a1[:], attn1T[:])

        # ---------------- o-proj + residual (generic) ----------------
        def out_proj(attn_T, w_dram, bias_idx, init_src, out_dram):
            # out = o_proj(attn) + bias + residual, written per column chunk
            for dc4 in range(4):
                w_t = wmed.tile([P, KT, 256], bf16, tag="med4")
                nc.sync.dma_start(w_t[:], w_dram[:, :, dc4 * 256:(dc4 + 1) * 256])
                bt = xpool.tile([P, 256], f32, tag="x")
                nc.sync.dma_start(bt[:], bias3_d[:, bias_idx, dc4 * 256:(dc4 + 1) * 256])
                for tt in range(TT_OWN):
                    ps = ps_a.tile([P, 512], f32, tag="psa")
                    for m in range(KT):
                        nc.tensor.matmul(ps[:, 0:256],
                                         attn_T[:, m, tt * P:(tt + 1) * P],
                                         w_t[:, m, :],
                                         start=(m == 0), stop=(m == KT - 1))
                    rt = xpool.tile([P, 256], f32, tag="x")
                    nc.sync.dma_start(rt[:], init_src[tt, :, dc4 * 256:(dc4 + 1) * 256])
                    tmp = evict.tile([P, 256], f32, tag="xn")
                    nc.vector.tensor_tensor(tmp[:], ps[:, 0:256], bt[:], ALU.add)
                    tmp2 = evict.tile([P, 256], f32, tag="xn2")
                    nc.vector.tensor_tensor(tmp2[:], tmp[:], rt[:], ALU.add)
                    nc.sync.dma_start(out_dram[tt, :, dc4 * 256:(dc4 + 1) * 256],
                                      tmp2[:])

        out_proj(attn1T, wo1_d, 0, xf_d, x1_dram)

        # ---------------- Phase 5: LN2 -> h2T; q2 ----------------
        h2T = expp.tile([P, KT, N_OWN], bf16, tag="expT")
        for tt in range(TT_OWN):
            xt = xpool.tile([P, D], f32, tag="x")
            nc.sync.dma_start(xt[:], x1_dram[tt])
            layernorm_tile(xt, tt, 1, dst_sb=h2T)

        q2T = big.tile([P, KT, N_OWN], bf16, tag="qT")
        w_sb = wres.tile([P, KT, INNER], bf16, tag="wbig")
        nc.sync.dma_start(w_sb[:], wq2_d[:])
        for m in range(KT):
            for qc in range(2):
                ps = ps_a.tile([P, 512], f32, tag="psa")
                for kt in range(KT):
                    nc.tensor.matmul(ps[:], w_sb[:, kt, m * P:(m + 1) * P],
                                     h2T[:, kt, qc * 512:(qc + 1) * 512],
                                     start=(kt == 0), stop=(kt == KT - 1))
                nc.vector.tensor_scalar_mul(q2T[:, m, qc * 512:(qc + 1) * 512],
                                            ps[:], SC)

        # ---------------- Phase 6: cross-attention ----------------
        k2T = smk.tile([P, KT, J], bf16, tag="k2T")
        w_sb = wres.tile([P, CKT, INNER], bf16, tag="wbig")
        nc.sync.dma_start(w_sb[:], wk2_d[:])
        for m in range(KT):
            ps = ps_a.tile([P, 512], f32, tag="psa")
            for kt in range(CKT):
                nc.tensor.matmul(ps[:, 0:J], w_sb[:, kt, m * P:(m + 1) * P],
                                 ctxT_sb[:, kt, :],
                                 start=(kt == 0), stop=(kt == CKT - 1))
            nc.vector.tensor_copy(k2T[:, m, :], ps[:, 0:J])
        v2_sb = smk.tile([P, J // P, H, DH + 1], bf16, tag="v2")
        nc.vector.memset(v2_sb[:], 1.0)
        w_sb = wres.tile([P, CKT, INNER], bf16, tag="wbig")
        nc.sync.dma_start(w_sb[:], wv2_d[:])
        for tt in range(J // P):
            for nc2 in range(2):
                ps = ps_a.tile([P, 512], f32, tag="psa")
                for kt in range(CKT):
                    nc.tensor.matmul(ps[:], ctxT_sb[:, kt, tt * P:(tt + 1) * P],
                                     w_sb[:, kt, nc2 * 512:(nc2 + 1) * 512],
                                     start=(kt == 0), stop=(kt == CKT - 1))
                nc.vector.tensor_copy(
                    v2_sb[:, tt, nc2 * 8:(nc2 + 1) * 8, 0:DH],
                    ps[:].rearrange("p (hh r) -> p hh r", r=DH))

        attn2T = big.tile([P, KT, N_OWN], bf16, tag="attnT")

        def get_k_cross(h):
            return k2T[:, h // 2, :]

        attention(get_k_cross, v2_sb, q2T, J // P, attn2T)

        out_proj(attn2T, wo2_d, 1, x1_dram, x2_dram)

        # ---------------- Phase 8: LN3 -> h3T ----------------
        h3T = big.tile([P, KT, N_OWN], bf16, tag="qT")
        for tt in range(TT_OWN):
            xt = xpool.tile([P, D], f32, tag="x")
            nc.sync.dma_start(xt[:], x2_dram[tt])
            layernorm_tile(xt, tt, 2, dst_sb=h3T)

        # ---------------- Phase 9: GEGLU FF ----------------
        g_sb = big.tile([P, 8, N_OWN], bf16, tag="attnT")
        y_sb = big.tile([P, TT_OWN, D], f32, tag="v33")
        for grp in range(4):
            wf2g = wres.tile([P, 8, D], bf16, tag="wbig")
            nc.sync.dma_start(wf2g[:], wf2_d[:, grp * 8:(grp + 1) * 8, :])
            for j in range(8):
                f = grp * 8 + j
                wa = wsm.tile([P, KT, P], bf16, tag="wstream")
                nc.sync.dma_start(wa[:], wf1_d[:, :, f * P:(f + 1) * P])
                wg = wsm.tile([P, KT, P], bf16, tag="wstream")
                nc.sync.dma_start(wg[:], wf1_d[:, :, FF + f * P:FF + (f + 1) * P])
                a_sb = evict.tile([P, N_OWN], bf16, tag="a_sb")
                gt_sb = evict.tile([P, N_OWN], bf16, tag="gt_sb")
                for qc in range(2):
                    sl = slice(qc * 512, (qc + 1) * 512)
                    ps1 = ps_sc.tile([P, 1024], f32, tag="pssc")
                    for kt in range(KT):
                        nc.tensor.matmul(ps1[:, 0:512], wa[:, kt, :], h3T[:, kt, sl],
                                         start=(kt == 0), stop=(kt == KT - 1))
                    nc.vector.tensor_scalar(a_sb[:, sl], ps1[:, 0:512],
                                            fb1_sb[:, f:f + 1], None, ALU.add)
                    ps2 = ps_sc.tile([P, 1024], f32, tag="pssc")
                    for kt in range(KT):
                        nc.tensor.matmul(ps2[:, 0:512], wg[:, kt, :], h3T[:, kt, sl],
                                         start=(kt == 0), stop=(kt == KT - 1))
                    nc.scalar.activation(gt_sb[:, sl], ps2[:, 0:512], AF.Gelu,
                                         bias=fb1_sb[:, 32 + f:32 + f + 1])
                nc.vector.tensor_tensor(g_sb[:, j, :], a_sb[:], gt_sb[:], ALU.mult)
            for tt in range(TT_OWN):
                for dc in range(2):
                    ps = ps_a.tile([P, 512], f32, tag="psa")
                    for j in range(8):
                        nc.tensor.matmul(ps[:], g_sb[:, j, tt * P:(tt + 1) * P],
                                         wf2g[:, j, dc * 512:(dc + 1) * 512],
                                         start=(j == 0), stop=(j == 7))
                    if grp == 0:
                        nc.vector.tensor_copy(
                            y_sb[:, tt, dc * 512:(dc + 1) * 512], ps[:])
                    else:
                        nc.vector.tensor_tensor(
                            y_sb[:, tt, dc * 512:(dc + 1) * 512],
                            y_sb[:, tt, dc * 512:(dc + 1) * 512], ps[:], ALU.add)
        # final: y = y_acc + ff_b2 + x2
        b2t = xpool.tile([P, D], f32, tag="x")
        nc.sync.dma_start(b2t[:], bias3_d[:, 2, :])
        for tt in range(TT_OWN):
            x2t = xpool.tile([P, D], f32, tag="x")
            nc.sync.dma_start(x2t[:], x2_dram[tt])
            yt = evict.tile([P, D], f32, tag="yt")
            nc.vector.tensor_tensor(yt[:], y_sb[:, tt, :], b2t[:], ALU.add)
            nc.vector.tensor_tensor(yt[:], yt[:], x2t[:], ALU.add)
            nc.sync.dma_start(y_d[tt], yt[:])

    nc.compile()
    return nc


def _rearr_w(w, kt):
    return np.ascontiguousarray(
        w.reshape(kt, P, -1).transpose(1, 0, 2)).astype(BF16)


def _shard_inputs(inputs):
    f = {k: np.asarray(v, dtype=np.float32) for k, v in inputs.items()}
    shared = {
        "nw": _rearr_w(np.concatenate([f["n1_w"], f["n2_w"], f["n3_w"]], axis=1), KT),
        "nbc": np.ascontiguousarray(
            np.concatenate([f["n1_b"], f["n2_b"], f["n3_b"]])
            .reshape(3, 16, P).transpose(2, 0, 1).reshape(P, 48)),
        "wq1": _rearr_w(f["q1"], KT), "wk1": _rearr_w(f["k1"], KT),
        "wv1": _rearr_w(f["v1"], KT), "wo1": _rearr_w(f["o1_w"], KT),
        "wq2": _rearr_w(f["q2"], KT), "wk2": _rearr_w(f["k2"], CKT),
        "wv2": _rearr_w(f["v2"], CKT), "wo2": _rearr_w(f["o2_w"], KT),
        "bias3": np.ascontiguousarray(np.broadcast_to(
            np.stack([f["o1_b"], f["o2_b"], f["ff_b2"]])[None], (P, 3, D))),
        "fb1c": np.ascontiguousarray(f["ff_b1"].reshape(64, P).T),
        "wf1": _rearr_w(f["ff_w1"], KT),
        "wf2": _rearr_w(f["ff_w2"], FF // P),
    }
    in_maps = []
    for core in range(8):
        b, half = core // 2, core % 2
        own = f["x"][b, half * N_OWN:(half + 1) * N_OWN]
        oth = f["x"][b, (1 - half) * N_OWN:(2 - half) * N_OWN]
        m = dict(shared)
        m["xf"] = np.ascontiguousarray(
            np.concatenate([own, oth]).reshape(TT_FULL, P, D))
        m["tT"] = np.ascontiguousarray(f["t"][b, 0].reshape(KT, P).T).astype(BF16)
        m["ctxT"] = np.ascontiguousarray(
            f["context"][b].T.reshape(CKT, P, J).transpose(1, 0, 2)).astype(BF16)
        in_maps.append(m)
    return in_maps


def kernel(**inputs):
    from concourse.bass_utils import run_bass_kernel_spmd
    if "nc" not in _CACHE:
        _CACHE["nc"] = _build_program()
    nc = _CACHE["nc"]
    in_maps = _shard_inputs(inputs)
    res = run_bass_kernel_spmd(nc, in_maps, core_ids=list(range(8)))
    out = np.empty((B, N, D), dtype=np.float32)
    for core in range(8):
        b, half = core // 2, core % 2
        out[b, half * N_OWN:(half + 1) * N_OWN] = \
            res.results[core]["y"].reshape(N_OWN, D)
    return out



# revision 10
# speedup vs baseline: 1.8003x; 1.8003x over previous
"""Trainium2 Bass kernel for nn_BasicTransformerBlock_35304631173827.

Sharding: 8 cores = 4 samples x 2 sequence halves. Each core computes its
1024-token half of one sample fully locally (self-attention K/V recomputed
over the full 2048-token sample -> zero collectives). bf16 matmuls with
fp32 PSUM accumulation; LayerNorm stats, softmax and residuals in fp32.

v2 restructure vs baseline:
- h1T kept SBUF-resident; dense projections use paired 512-col PSUM chains.
- Attention: every head's AV matmul is M=65 with a ones-column in V so the
  softmax denominator falls out of the same accumulation chain (no separate
  M=1 denominator matmuls); reciprocal_approx_fast for 1/den; odd heads'
  outputs shifted to partitions 64:128 via a small SBUF->SBUF DMA.
- qc-outer / head-inner loop with o-proj + LN interleaved to keep the PE
  array busy (p-state ramp) while the scalar engine chews softmax exps.
- FF: PSUM-accumulated FF2 (full K=4096 contraction in one chain), fused
  (a+b1)*gelu(gate+b1') via scalar_tensor_tensor, output biases folded into
  the matmul chains as K=1 ones-row accumulation steps.
"""

import numpy as np
import ml_dtypes

BF16 = ml_dtypes.bfloat16

B, N, D = 4, 2048, 1024
J, CD = 256, 768
H, DH = 16, 64
INNER = 1024
FF = 4096
P = 128
KT = D // P            # 8
CKT = CD // P          # 6
TT_FULL = N // P       # 16
N_OWN = N // 2
TT_OWN = N_OWN // P    # 8
EPS = 1e-5
SC = DH ** -0.5

_CACHE = {}


def _build_program():
    import concourse.tile as tile
    from concourse import mybir, bacc
    from concourse.masks import make_identity
    from contextlib import ExitStack

    f32 = mybir.dt.float32
    bf16 = mybir.dt.bfloat16
    AF = mybir.ActivationFunctionType
    ALU = mybir.AluOpType

    nc = bacc.Bacc(None, target_bir_lowering=False)

    xf_d = nc.dram_tensor("xf", [TT_FULL, P, D], f32, kind="ExternalInput")
    tT_d = nc.dram_tensor("tT", [P, KT], bf16, kind="ExternalInput")
    nw_d = nc.dram_tensor("nw", [P, KT, 6 * D], bf16, kind="ExternalInput")
    nbc_d = nc.dram_tensor("nbc", [P, 48], f32, kind="ExternalInput")
    wq1_d = nc.dram_tensor("wq1", [P, KT, INNER], bf16, kind="ExternalInput")
    wk1_d = nc.dram_tensor("wk1", [P, KT, INNER], bf16, kind="ExternalInput")
    wv1_d = nc.dram_tensor("wv1", [P, KT, INNER], bf16, kind="ExternalInput")
    wo1_d = nc.dram_tensor("wo1", [P, KT, D], bf16, kind="ExternalInput")
    wq2_d = nc.dram_tensor("wq2", [P, KT, INNER], bf16, kind="ExternalInput")
    wk2_d = nc.dram_tensor("wk2", [P, CKT, INNER], bf16, kind="ExternalInput")
    wv2_d = nc.dram_tensor("wv2", [P, CKT, INNER], bf16, kind="ExternalInput")
    wo2_d = nc.dram_tensor("wo2", [P, KT, D], bf16, kind="ExternalInput")
    ctxT_d = nc.dram_tensor("ctxT", [P, CKT, J], bf16, kind="ExternalInput")
    brow_d = nc.dram_tensor("brow", [1, 3 * D], bf16, kind="ExternalInput")
    fb1_d = nc.dram_tensor("fb1c", [P, 64], f32, kind="ExternalInput")
    wf1_d = nc.dram_tensor("wf1", [P, KT, 2 * FF], bf16, kind="ExternalInput")
    wf2_d = nc.dram_tensor("wf2", [P, FF // P, D], bf16, kind="ExternalInput")
    y_d = nc.dram_tensor("y", [TT_OWN, P, D], f32, kind="ExternalOutput")

    # DRAM scratch
    kT_dram = nc.dram_tensor("scr_kT", [KT, P, N], bf16, kind="Internal")
    x1_dram = nc.dram_tensor("scr_x1", [TT_OWN, P, D], f32, kind="Internal")
    x2_dram = nc.dram_tensor("scr_x2", [TT_OWN, P, D], f32, kind="Internal")

    with tile.TileContext(nc) as tc, ExitStack() as es:
        konst = es.enter_context(tc.tile_pool(name="konst", bufs=1))
        xpool = es.enter_context(tc.tile_pool(name="xpool", bufs=2))
        stats = es.enter_context(tc.tile_pool(name="stats", bufs=3))
        small = es.enter_context(tc.tile_pool(name="small", bufs=2))
        wsm = es.enter_context(tc.tile_pool(name="wsm", bufs=4))
        wbig = es.enter_context(tc.tile_pool(name="wbig", bufs=2))
        stg = es.enter_context(tc.tile_pool(name="stg", bufs=2))
        hTp = es.enter_context(tc.tile_pool(name="hTp", bufs=1))
        ps_big = es.enter_context(tc.tile_pool(name="ps_big", bufs=2, space="PSUM"))
        ps_av = es.enter_context(tc.tile_pool(name="ps_av", bufs=2, space="PSUM"))
        ps_bc = es.enter_context(tc.tile_pool(name="ps_bc", bufs=2, space="PSUM"))

        # ---------------- constants ----------------
        ident = konst.tile([P, P], bf16)
        make_identity(nc, ident)
        ones = konst.tile([1, P], bf16)
        nc.vector.memset(ones[:], 1.0)
        eps_t = konst.tile([P, 1], f32)
        nc.vector.memset(eps_t[:], EPS)
        tT_sb = konst.tile([P, KT], bf16)
        nc.sync.dma_start(tT_sb[:], tT_d[:])
        nbc_sb = konst.tile([P, 48], f32)
        nc.sync.dma_start(nbc_sb[:], nbc_d[:])
        fb1_sb = konst.tile([P, 64], f32)
        nc.sync.dma_start(fb1_sb[:], fb1_d[:])
        ctxT_sb = konst.tile([P, CKT, J], bf16)
        nc.sync.dma_start(ctxT_sb[:], ctxT_d[:])
        brow_sb = konst.tile([1, 3 * D], bf16)
        nc.sync.dma_start(brow_sb[:], brow_d[:])
        cols = konst.tile([P, 48], f32)

        # ---------------- Phase 0: AdaLN embeddings ----------------
        # emb^T chunks: cols[:, cc] = (t @ nW)[cc*128 : (cc+1)*128]
        for g in range(3):
            for c in range(16):
                cc = g * 16 + c
                nwt = wsm.tile([P, KT, P], bf16, tag="wstream")
                nc.sync.dma_start(nwt[:], nw_d[:, :, cc * P:(cc + 1) * P])
                ps = ps_av.tile([P, 512], f32, tag="av")
                for kt in range(KT):
                    nc.tensor.matmul(ps[:, 0:1], nwt[:, kt, :], tT_sb[:, kt:kt + 1],
                                     start=(kt == 0), stop=(kt == KT - 1))
                nc.vector.tensor_copy(cols[:, cc:cc + 1], ps[:, 0:1])
            sl = slice(g * 16, g * 16 + 16)
            nc.vector.tensor_add(cols[:, sl], cols[:, sl], nbc_sb[:, sl])
            nc.vector.tensor_scalar_add(cols[:, g * 16:g * 16 + 8],
                                        cols[:, g * 16:g * 16 + 8], 1.0)

        def layernorm_tile(x_tile, n3, dst_sb, off):
            """LayerNorm + AdaLN affine on (P, D) tile -> transposed chunks
            written to dst_sb[:, c, off:off+128]."""
            bst = stats.tile([P, 2, 6], f32, tag="bnst")
            for g in range(2):
                nc.vector.bn_stats(bst[:, g, :], x_tile[:, g * 512:(g + 1) * 512])
            mv = stats.tile([P, 4], f32, tag="mv")
            nc.vector.bn_aggr(mv[:, 0:2], bst[:])
            nc.scalar.activation(mv[:, 2:3], mv[:, 1:2], AF.Sqrt, bias=eps_t[:])
            nc.vector.reciprocal(mv[:, 2:3], mv[:, 2:3])
            nc.vector.tensor_tensor(mv[:, 3:4], mv[:, 0:1], mv[:, 2:3], ALU.mult)
            nc.vector.tensor_scalar_mul(mv[:, 3:4], mv[:, 3:4], -1.0)
            xn = stats.tile([P, D], bf16, tag="xn")
            nc.scalar.activation(xn[:], x_tile[:], AF.Identity,
                                 bias=mv[:, 3:4], scale=mv[:, 2:3])
            for c in range(KT):
                pt = ps_bc.tile([P, P], bf16, tag="bc")
                nc.tensor.transpose(pt[:], xn[:, c * P:(c + 1) * P], ident[:])
                nc.vector.tensor_scalar(
                    dst_sb[:, c, off:off + P], pt[:],
                    cols[:, n3 * 16 + c:n3 * 16 + c + 1],
                    cols[:, n3 * 16 + 8 + c:n3 * 16 + 8 + c + 1],
                    ALU.mult, ALU.add)

        # outer-attention scope: qT/q2T, v tiles
        with tc.tile_pool(name="qTp", bufs=1) as qTp, \
             tc.tile_pool(name="vp", bufs=1) as vp:

            # ---------------- Phase 1+2: LN1 -> h1T (SBUF); QKV ----------------
            with tc.tile_pool(name="h1p", bufs=1) as h1p:
                h1T = h1p.tile([P, KT, N], bf16, tag="h1T")
                for tt in range(TT_FULL):
                    xt = xpool.tile([P, D], f32, tag="x")
                    nc.sync.dma_start(xt[:], xf_d[tt])
                    layernorm_tile(xt, 0, h1T, tt * P)

                # Q projection (own half, softmax scale pre-folded into wq1)
                qT = qTp.tile([P, KT, N_OWN], bf16, tag="qT")
                w_sb = wbig.tile([P, KT, INNER], bf16, tag="w")
                nc.sync.dma_start(w_sb[:], wq1_d[:])
                for m in range(KT):
                    ps = ps_big.tile([P, 1024], f32, tag="big")
                    for qc in range(2):
                        for kt in range(KT):
                            nc.tensor.matmul(
                                ps[:, qc * 512:(qc + 1) * 512],
                                w_sb[:, kt, m * P:(m + 1) * P],
                                h1T[:, kt, qc * 512:(qc + 1) * 512],
                                start=(kt == 0), stop=(kt == KT - 1))
                    nc.vector.tensor_copy(qT[:, m, :], ps[:])

                # K projection (full sample) -> DRAM
                w_sb = wbig.tile([P, KT, INNER], bf16, tag="w")
                nc.sync.dma_start(w_sb[:], wk1_d[:])
                for m in range(KT):
                    for half in range(2):
                        ps = ps_big.tile([P, 1024], f32, tag="big")
                        for qc in range(2):
                            for kt in range(KT):
                                nc.tensor.matmul(
                                    ps[:, qc * 512:(qc + 1) * 512],
                                    w_sb[:, kt, m * P:(m + 1) * P],
                                    h1T[:, kt, half * 1024 + qc * 512:
                                        half * 1024 + (qc + 1) * 512],
                                    start=(kt == 0), stop=(kt == KT - 1))
                        kst = stg.tile([P, 1024], bf16, tag="kst")
                        nc.vector.tensor_copy(kst[:], ps[:])
                        nc.sync.dma_start(
                            kT_dram[m, :, half * 1024:(half + 1) * 1024], kst[:])

                # V projection (full sample), ones column at index DH
                v_sb = vp.tile([P, TT_FULL, H, DH + 1], bf16, tag="v1")
                nc.vector.memset(v_sb[:], 1.0)
                w_sb = wbig.tile([P, KT, INNER], bf16, tag="w")
                nc.sync.dma_start(w_sb[:], wv1_d[:])
                for tt in range(TT_FULL):
                    ps = ps_big.tile([P, 1024], f32, tag="big")
                    for nc2 in range(2):
                        for kt in range(KT):
                            nc.tensor.matmul(
                                ps[:, nc2 * 512:(nc2 + 1) * 512],
                                h1T[:, kt, tt * P:(tt + 1) * P],
                                w_sb[:, kt, nc2 * 512:(nc2 + 1) * 512],
                                start=(kt == 0), stop=(kt == KT - 1))
                    nc.vector.tensor_copy(
                        v_sb[:, tt, :, 0:DH],
                        ps[:].rearrange("p (hh r) -> p hh r", r=DH))

            # h1T freed here.

            # ---------------- cross K2/V2 (early, PE filler) ----------------
            k2T = vp.tile([P, KT, J], bf16, tag="k2T")
            w_sb = wbig.tile([P, KT, INNER], bf16, tag="w")
            nc.sync.dma_start(w_sb[:, 0:CKT, :], wk2_d[:])
            for m in range(KT):
                ps = ps_av.tile([P, 512], f32, tag="av")
                for kt in range(CKT):
                    nc.tensor.matmul(ps[:, 0:J], w_sb[:, kt, m * P:(m + 1) * P],
                                     ctxT_sb[:, kt, :],
                                     start=(kt == 0), stop=(kt == CKT - 1))
                nc.vector.tensor_copy(k2T[:, m, :], ps[:, 0:J])

            v2_sb = vp.tile([P, J // P, H, DH + 1], bf16, tag="v2")
            nc.vector.memset(v2_sb[:], 1.0)
            w_sb = wbig.tile([P, KT, INNER], bf16, tag="w")
            nc.sync.dma_start(w_sb[:, 0:CKT, :], wv2_d[:])
            for tt in range(J // P):
                ps = ps_big.tile([P, 1024], f32, tag="big")
                for nc2 in range(2):
                    for kt in range(CKT):
                        nc.tensor.matmul(
                            ps[:, nc2 * 512:(nc2 + 1) * 512],
                            ctxT_sb[:, kt, tt * P:(tt + 1) * P],
                            w_sb[:, kt, nc2 * 512:(nc2 + 1) * 512],
                            start=(kt == 0), stop=(kt == CKT - 1))
                nc.vector.tensor_copy(
                    v2_sb[:, tt, :, 0:DH],
                    ps[:].rearrange("p (hh r) -> p hh r", r=DH))

            # ---------------- attention core ----------------
            with tc.tile_pool(name="expp", bufs=2) as expp, \
                 tc.tile_pool(name="atp", bufs=1) as atp, \
                 tc.tile_pool(name="kcp", bufs=2) as kcp:

                def attn_head(h, qc, get_k, v_t, qT_t, nkt, out_T):
                    """One (head, query-chunk) of attention -> out_T slice."""
                    hp = (h % 2) * 64
                    m2 = h // 2
                    qs = slice(qc * 512, (qc + 1) * 512)
                    kap = get_k(h)
                    # scores + exp, 2 key-tiles per PSUM
                    exs = []
                    for half in range((nkt + 7) // 8):
                        ex = expp.tile([P, 8, 512], bf16, tag="ex")
                        exs.append(ex)
                    for kt2 in range((nkt + 1) // 2):
                        ps = ps_big.tile([P, 1024], f32, tag="big")
                        for u in range(min(2, nkt)):
                            kt = kt2 * 2 + u
                            nc.tensor.matmul(
                                ps[:, u * 512:(u + 1) * 512],
                                kap[hp:hp + 64, kt * P:(kt + 1) * P],
                                qT_t[hp:hp + 64, m2, qs],
                                start=True, stop=True)
                        nkk = min(2, nkt)
                        ex = exs[kt2 // 4]
                        lo = (kt2 % 4) * 2
                        nc.scalar.activation(
                            ex[:, lo:lo + nkk, :].rearrange("p a b -> p (a b)"),
                            ps[:, 0:nkk * 512], AF.Exp)
                    # AV with denominator from the ones column of v:
                    # pav[0:64] = data, pav[64] = softmax denominator
                    pav = ps_av.tile([P, 512], f32, tag="av")
                    for kt in range(nkt):
                        nc.tensor.matmul(
                            pav[0:65], v_t[:, kt, h, :],
                            exs[kt // 8][:, kt % 8, :],
                            start=(kt == 0), stop=(kt == nkt - 1))
                    den = small.tile([1, 512], f32, tag="den")
                    nc.vector.tensor_copy(den[:], pav[64:65, :])
                    rec32 = small.tile([1, 512], f32, tag="rec32")
                    nc.vector.reciprocal_approx_fast(rec32[:], den[:])
                    bcs = small.tile([64, 512], f32, tag="bcs")
                    nc.gpsimd.partition_broadcast(bcs[:], rec32[:])
                    if hp == 0:
                        nc.vector.tensor_tensor(out_T[0:64, m2, qs],
                                                pav[0:64], bcs[:], ALU.mult)
                    else:
                        tmp = small.tile([64, 512], bf16, tag="todd")
                        nc.vector.tensor_tensor(tmp[:], pav[0:64],
                                                bcs[:], ALU.mult)
                        nc.sync.dma_start(out_T[64:128, m2, qs], tmp[:])

                def out_proj(attn_T, w_t, brow_i, resid_src, out_dram, tt,
                             ln_grp, h_dst):
                    """o-proj + bias + residual for token tile tt; LN into
                    h_dst."""
                    ps = ps_big.tile([P, 1024], f32, tag="big")
                    for dc in range(2):
                        dsl = slice(dc * 512, (dc + 1) * 512)
                        for m in range(KT):
                            nc.tensor.matmul(ps[:, dsl],
                                             attn_T[:, m, tt * P:(tt + 1) * P],
                                             w_t[:, m, dsl],
                                             start=(m == 0), stop=False)
                        nc.tensor.matmul(
                            ps[:, dsl], ones[0:1, :],
                            brow_sb[0:1, brow_i * D + dc * 512:
                                    brow_i * D + (dc + 1) * 512],
                            start=False, stop=True)
                    rt = xpool.tile([P, D], f32, tag="x")
                    nc.sync.dma_start(rt[:], resid_src[tt])
                    xot = stg.tile([P, D], f32, tag="ostage")
                    nc.vector.tensor_tensor(xot[:], ps[:], rt[:], ALU.add)
                    nc.sync.dma_start(out_dram[tt], xot[:])
                    layernorm_tile(xot, ln_grp, h_dst, tt * P)

                # -------- self-attention + o1 + LN2, qc-interleaved --------
                attn1T = atp.tile([P, KT, N_OWN], bf16, tag="attnT")
                h2T = hTp.tile([P, KT, N_OWN], bf16, tag="hT")
                wo1_sb = wbig.tile([P, KT, INNER], bf16, tag="w")
                nc.sync.dma_start(wo1_sb[:], wo1_d[:])

                _kc = {}

                def get_k_self(h):
                    m2 = h // 2
                    if _kc.get("m2") != m2:
                        kth = kcp.tile([P, N], bf16, tag="kth")
                        nc.sync.dma_start(kth[:], kT_dram[m2])
                        _kc["m2"] = m2
                        _kc["t"] = kth
                    return _kc["t"]

                for qc in range(2):
                    _kc.clear()
                    for h in range(H):
                        attn_head(h, qc, get_k_self, v_sb, qT, TT_FULL, attn1T)
                    for tt in range(qc * 4, qc * 4 + 4):
                        out_proj(attn1T, wo1_sb, 0, xf_d, x1_dram, tt, 1, h2T)
                    if qc == 0:
                        # prefetch q2 weights mid-attention (ring slot of wv2)
                        wq2_sb = wbig.tile([P, KT, INNER], bf16, tag="w")
                        nc.sync.dma_start(wq2_sb[:], wq2_d[:])

                # -------- q2 projection --------
                q2T = qTp.tile([P, KT, N_OWN], bf16, tag="qT")
                for m in range(KT):
                    ps = ps_big.tile([P, 1024], f32, tag="big")
                    for qc in range(2):
                        for kt in range(KT):
                            nc.tensor.matmul(
                                ps[:, qc * 512:(qc + 1) * 512],
                                wq2_sb[:, kt, m * P:(m + 1) * P],
                                h2T[:, kt, qc * 512:(qc + 1) * 512],
                                start=(kt == 0), stop=(kt == KT - 1))
                    nc.vector.tensor_copy(q2T[:, m, :], ps[:])

                # -------- cross-attention + o2 + LN3 --------
                attn2T = atp.tile([P, KT, N_OWN], bf16, tag="attnT")
                h3T = hTp.tile([P, KT, N_OWN], bf16, tag="hT")
                wo2_sb = wbig.tile([P, KT, INNER], bf16, tag="w")
                nc.sync.dma_start(wo2_sb[:], wo2_d[:])

                def get_k_cross(h):
                    return k2T[:, h // 2, :]

                for qc in range(2):
                    for h in range(H):
                        attn_head(h, qc, get_k_cross, v2_sb, q2T, J // P, attn2T)
                    for tt in range(qc * 4, qc * 4 + 4):
                        out_proj(attn2T, wo2_sb, 1, x1_dram, x2_dram, tt, 2, h3T)

        # ---------------- Phase 6: GEGLU FF ----------------
        with tc.tile_pool(name="gp", bufs=1) as gp, \
             tc.tile_pool(name="wf2p", bufs=1) as wf2p:
            g_sb = gp.tile([P, 32, N_OWN], bf16, tag="g")
            for fc in range(32):
                wa = wsm.tile([P, KT, P], bf16, tag="wstream")
                nc.sync.dma_start(wa[:], wf1_d[:, :, fc * P:(fc + 1) * P])
                wg = wsm.tile([P, KT, P], bf16, tag="wstream")
                nc.sync.dma_start(wg[:], wf1_d[:, :, FF + fc * P:FF + (fc + 1) * P])
                for qc in range(2):
                    qs = slice(qc * 512, (qc + 1) * 512)
                    ps = ps_big.tile([P, 1024], f32, tag="big")
                    for kt in range(KT):
                        nc.tensor.matmul(ps[:, 0:512], wa[:, kt, :], h3T[:, kt, qs],
                                         start=(kt == 0), stop=(kt == KT - 1))
                    for kt in range(KT):
                        nc.tensor.matmul(ps[:, 512:1024], wg[:, kt, :],
                                         h3T[:, kt, qs],
                                         start=(kt == 0), stop=(kt == KT - 1))
                    gt = small.tile([P, 512], bf16, tag="gt")
                    nc.scalar.activation(gt[:], ps[:, 512:1024], AF.Gelu,
                                         bias=fb1_sb[:, 32 + fc:32 + fc + 1])
                    nc.vector.scalar_tensor_tensor(
                        g_sb[:, fc, qs], ps[:, 0:512], fb1_sb[:, fc:fc + 1],
                        gt[:], ALU.add, ALU.mult)

            for dc in range(2):
                dsl = slice(dc * 512, (dc + 1) * 512)
                wf2t = wf2p.tile([P, 32, 512], bf16, tag="wf2")
                nc.sync.dma_start(wf2t[:], wf2_d[:, :, dsl])
                for tt in range(TT_OWN):
                    ps = ps_av.tile([P, 512], f32, tag="av")
                    for j in range(32):
                        nc.tensor.matmul(ps[:], g_sb[:, j, tt * P:(tt + 1) * P],
                                         wf2t[:, j, :],
                                         start=(j == 0), stop=False)
                    nc.tensor.matmul(ps[:], ones[0:1, :],
                                     brow_sb[0:1, 2 * D + dc * 512:
                                             2 * D + (dc + 1) * 512],
                                     start=False, stop=True)
                    rt = xpool.tile([P, D], f32, tag="x")
                    nc.sync.dma_start(rt[:, 0:512], x2_dram[tt, :, dsl])
                    yt = stg.tile([P, D], f32, tag="ostage")
                    nc.vector.tensor_tensor(yt[:, 0:512], ps[:], rt[:, 0:512],
                                            ALU.add)
                    nc.sync.dma_start(y_d[tt, :, dsl], yt[:, 0:512])

    nc.compile()
    return nc


def _rearr_w(w, kt):
    return np.ascontiguousarray(
        w.reshape(kt, P, -1).transpose(1, 0, 2)).astype(BF16)


def _shard_inputs(inputs):
    f = {k: np.asarray(v, dtype=np.float32) for k, v in inputs.items()}
    shared = {
        "nw": _rearr_w(np.concatenate([f["n1_w"], f["n2_w"], f["n3_w"]], axis=1), KT),
        "nbc": np.ascontiguousarray(
            np.concatenate([f["n1_b"], f["n2_b"], f["n3_b"]])
            .reshape(3, 16, P).transpose(2, 0, 1).reshape(P, 48)),
        "wq1": _rearr_w(f["q1"] * SC, KT), "wk1": _rearr_w(f["k1"], KT),
        "wv1": _rearr_w(f["v1"], KT), "wo1": _rearr_w(f["o1_w"], KT),
        "wq2": _rearr_w(f["q2"] * SC, KT), "wk2": _rearr_w(f["k2"], CKT),
        "wv2": _rearr_w(f["v2"], CKT), "wo2": _rearr_w(f["o2_w"], KT),
        "brow": np.ascontiguousarray(
            np.concatenate([f["o1_b"], f["o2_b"], f["ff_b2"]])
            .reshape(1, 3 * D)).astype(BF16),
        "fb1c": np.ascontiguousarray(f["ff_b1"].reshape(64, P).T),
        "wf1": _rearr_w(f["ff_w1"], KT),
        "wf2": _rearr_w(f["ff_w2"], FF // P),
    }
    in_maps = []
    for core in range(8):
        b, half = core // 2, core % 2
        own = f["x"][b, half * N_OWN:(half + 1) * N_OWN]
        oth = f["x"][b, (1 - half) * N_OWN:(2 - half) * N_OWN]
        m = dict(shared)
        m["xf"] = np.ascontiguousarray(
            np.concatenate([own, oth]).reshape(TT_FULL, P, D))
        m["tT"] = np.ascontiguousarray(f["t"][b, 0].reshape(KT, P).T).astype(BF16)
        m["ctxT"] = np.ascontiguousarray(
            f["context"][b].T.reshape(CKT, P, J).transpose(1, 0, 2)).astype(BF16)
        in_maps.append(m)
    return in_maps


def kernel(**inputs):
    from concourse.bass_utils import run_bass_kernel_spmd
    if "nc" not in _CACHE:
        _CACHE["nc"] = _build_program()
    nc = _CACHE["nc"]
    in_maps = _shard_inputs(inputs)
    res = run_bass_kernel_spmd(nc, in_maps, core_ids=list(range(8)))
    out = np.empty((B, N, D), dtype=np.float32)
    for core in range(8):
        b, half = core // 2, core % 2
        out[b, half * N_OWN:(half + 1) * N_OWN] = \
            res.results[core]["y"].reshape(N_OWN, D)
    return out


# revision 12
# speedup vs baseline: 1.8425x; 1.0234x over previous
"""Trainium2 Bass kernel for nn_BasicTransformerBlock_35304631173827.

Sharding: 8 cores = 4 samples x 2 sequence halves. Each core computes its
1024-token half of one sample fully locally (self-attention K/V recomputed
over the full 2048-token sample -> zero collectives). bf16 matmuls with
fp32 PSUM accumulation; LayerNorm stats, softmax and residuals in fp32.

v2 restructure vs baseline:
- h1T kept SBUF-resident; dense projections use paired 512-col PSUM chains.
- Attention: every head's AV matmul is M=65 with a ones-column in V so the
  softmax denominator falls out of the same accumulation chain (no separate
  M=1 denominator matmuls); reciprocal_approx_fast for 1/den; odd heads'
  outputs shifted to partitions 64:128 via a small SBUF->SBUF DMA.
- qc-outer / head-inner loop with o-proj + LN interleaved to keep the PE
  array busy (p-state ramp) while the scalar engine chews softmax exps.
- FF: PSUM-accumulated FF2 (full K=4096 contraction in one chain), fused
  (a+b1)*gelu(gate+b1') via scalar_tensor_tensor, output biases folded into
  the matmul chains as K=1 ones-row accumulation steps.
"""

import numpy as np
import ml_dtypes

BF16 = ml_dtypes.bfloat16

B, N, D = 4, 2048, 1024
J, CD = 256, 768
H, DH = 16, 64
INNER = 1024
FF = 4096
P = 128
KT = D // P            # 8
CKT = CD // P          # 6
TT_FULL = N // P       # 16
N_OWN = N // 2
TT_OWN = N_OWN // P    # 8
EPS = 1e-5
SC = DH ** -0.5

_CACHE = {}


def _build_program():
    import concourse.tile as tile
    from concourse import mybir, bacc
    from concourse.masks import make_identity
    from contextlib import ExitStack

    f32 = mybir.dt.float32
    bf16 = mybir.dt.bfloat16
    AF = mybir.ActivationFunctionType
    ALU = mybir.AluOpType

    nc = bacc.Bacc(None, target_bir_lowering=False)

    xf_d = nc.dram_tensor("xf", [TT_FULL, P, D], f32, kind="ExternalInput")
    tT_d = nc.dram_tensor("tT", [P, KT], bf16, kind="ExternalInput")
    nw_d = nc.dram_tensor("nw", [P, KT, 6 * D], bf16, kind="ExternalInput")
    nbc_d = nc.dram_tensor("nbc", [P, 48], f32, kind="ExternalInput")
    wq1_d = nc.dram_tensor("wq1", [P, KT, INNER], bf16, kind="ExternalInput")
    wk1_d = nc.dram_tensor("wk1", [P, KT, INNER], bf16, kind="ExternalInput")
    wv1_d = nc.dram_tensor("wv1", [P, KT, INNER], bf16, kind="ExternalInput")
    wo1_d = nc.dram_tensor("wo1", [P, KT, D], bf16, kind="ExternalInput")
    wq2_d = nc.dram_tensor("wq2", [P, KT, INNER], bf16, kind="ExternalInput")
    wk2_d = nc.dram_tensor("wk2", [P, CKT, INNER], bf16, kind="ExternalInput")
    wv2_d = nc.dram_tensor("wv2", [P, CKT, INNER], bf16, kind="ExternalInput")
    wo2_d = nc.dram_tensor("wo2", [P, KT, D], bf16, kind="ExternalInput")
    ctxT_d = nc.dram_tensor("ctxT", [P, CKT, J], bf16, kind="ExternalInput")
    brow_d = nc.dram_tensor("brow", [1, 3 * D], bf16, kind="ExternalInput")
    fb1_d = nc.dram_tensor("fb1c", [P, 64], f32, kind="ExternalInput")
    wf1_d = nc.dram_tensor("wf1", [P, KT, 2 * FF], bf16, kind="ExternalInput")
    wf2_d = nc.dram_tensor("wf2", [P, FF // P, D], bf16, kind="ExternalInput")
    y_d = nc.dram_tensor("y", [TT_OWN, P, D], f32, kind="ExternalOutput")

    # DRAM scratch
    kT_dram = nc.dram_tensor("scr_kT", [KT, P, N], bf16, kind="Internal")
    x1_dram = nc.dram_tensor("scr_x1", [TT_OWN, P, D], f32, kind="Internal")
    x2_dram = nc.dram_tensor("scr_x2", [TT_OWN, P, D], f32, kind="Internal")

    with tile.TileContext(nc) as tc, ExitStack() as es:
        konst = es.enter_context(tc.tile_pool(name="konst", bufs=1))
        xpool = es.enter_context(tc.tile_pool(name="xpool", bufs=2))
        stats = es.enter_context(tc.tile_pool(name="stats", bufs=3))
        small = es.enter_context(tc.tile_pool(name="small", bufs=2))
        wsm = es.enter_context(tc.tile_pool(name="wsm", bufs=4))
        wbig = es.enter_context(tc.tile_pool(name="wbig", bufs=2))
        stg = es.enter_context(tc.tile_pool(name="stg", bufs=2))
        hTp = es.enter_context(tc.tile_pool(name="hTp", bufs=1))
        ps_big = es.enter_context(tc.tile_pool(name="ps_big", bufs=2, space="PSUM"))
        ps_av = es.enter_context(tc.tile_pool(name="ps_av", bufs=2, space="PSUM"))
        ps_bc = es.enter_context(tc.tile_pool(name="ps_bc", bufs=2, space="PSUM"))

        # ---------------- constants ----------------
        ident = konst.tile([P, P], bf16)
        make_identity(nc, ident)
        ones = konst.tile([1, P], bf16)
        nc.vector.memset(ones[:], 1.0)
        eps_t = konst.tile([P, 1], f32)
        nc.vector.memset(eps_t[:], EPS)
        tT_sb = konst.tile([P, KT], bf16)
        nc.sync.dma_start(tT_sb[:], tT_d[:])
        nbc_sb = konst.tile([P, 48], f32)
        nc.sync.dma_start(nbc_sb[:], nbc_d[:])
        fb1_sb = konst.tile([P, 64], f32)
        nc.sync.dma_start(fb1_sb[:], fb1_d[:])
        ctxT_sb = konst.tile([P, CKT, J], bf16)
        nc.sync.dma_start(ctxT_sb[:], ctxT_d[:])
        brow_sb = konst.tile([1, 3 * D], bf16)
        nc.sync.dma_start(brow_sb[:], brow_d[:])
        cols = konst.tile([P, 48], f32)

        # ---------------- Phase 0: AdaLN embeddings ----------------
        # emb^T chunks: cols[:, cc] = (t @ nW)[cc*128 : (cc+1)*128]
        for g in range(3):
            for c in range(16):
                cc = g * 16 + c
                nwt = wsm.tile([P, KT, P], bf16, tag="wstream")
                nc.sync.dma_start(nwt[:], nw_d[:, :, cc * P:(cc + 1) * P])
                ps = ps_av.tile([P, 512], f32, tag="av")
                for kt in range(KT):
                    nc.tensor.matmul(ps[:, 0:1], nwt[:, kt, :], tT_sb[:, kt:kt + 1],
                                     start=(kt == 0), stop=(kt == KT - 1))
                nc.vector.tensor_copy(cols[:, cc:cc + 1], ps[:, 0:1])
            sl = slice(g * 16, g * 16 + 16)
            nc.vector.tensor_add(cols[:, sl], cols[:, sl], nbc_sb[:, sl])
            nc.vector.tensor_scalar_add(cols[:, g * 16:g * 16 + 8],
                                        cols[:, g * 16:g * 16 + 8], 1.0)

        def layernorm_tile(x_tile, n3, dst_sb, off):
            """LayerNorm + AdaLN affine on (P, D) tile -> transposed chunks
            written to dst_sb[:, c, off:off+128]."""
            bst = stats.tile([P, 2, 6], f32, tag="bnst")
            for g in range(2):
                nc.vector.bn_stats(bst[:, g, :], x_tile[:, g * 512:(g + 1) * 512])
            mv = stats.tile([P, 4], f32, tag="mv")
            nc.vector.bn_aggr(mv[:, 0:2], bst[:])
            nc.scalar.activation(mv[:, 2:3], mv[:, 1:2], AF.Sqrt, bias=eps_t[:])
            nc.vector.reciprocal(mv[:, 2:3], mv[:, 2:3])
            nc.vector.tensor_tensor(mv[:, 3:4], mv[:, 0:1], mv[:, 2:3], ALU.mult)
            nc.vector.tensor_scalar_mul(mv[:, 3:4], mv[:, 3:4], -1.0)
            xn = stats.tile([P, D], bf16, tag="xn")
            nc.scalar.activation(xn[:], x_tile[:], AF.Identity,
                                 bias=mv[:, 3:4], scale=mv[:, 2:3])
            for c in range(KT):
                pt = ps_bc.tile([P, P], bf16, tag="bc")
                nc.tensor.transpose(pt[:], xn[:, c * P:(c + 1) * P], ident[:])
                nc.vector.tensor_scalar(
                    dst_sb[:, c, off:off + P], pt[:],
                    cols[:, n3 * 16 + c:n3 * 16 + c + 1],
                    cols[:, n3 * 16 + 8 + c:n3 * 16 + 8 + c + 1],
                    ALU.mult, ALU.add)

        # outer-attention scope: qT/q2T, v tiles
        with tc.tile_pool(name="qTp", bufs=1) as qTp, \
             tc.tile_pool(name="vp", bufs=1) as vp:

            # ---------------- Phase 1+2: LN1 -> h1T (SBUF); QKV ----------------
            # LN1 interleaved with V-projection in 4-tile token groups so the
            # PE array has matmul work while LN chains stream through DVE.
            with tc.tile_pool(name="h1p", bufs=1) as h1p:
                h1T = h1p.tile([P, KT, N], bf16, tag="h1T")
                v_sb = vp.tile([P, TT_FULL, H, DH + 1], bf16, tag="v1")
                nc.vector.memset(v_sb[:], 1.0)
                wv_sb = wbig.tile([P, KT, INNER], bf16, tag="w")
                nc.sync.dma_start(wv_sb[:], wv1_d[:])
                for grp in range(4):
                    for tt in range(grp * 4, grp * 4 + 4):
                        xt = xpool.tile([P, D], f32, tag="x")
                        nc.sync.dma_start(xt[:], xf_d[tt])
                        layernorm_tile(xt, 0, h1T, tt * P)
                    for tt in range(grp * 4, grp * 4 + 4):
                        ps = ps_big.tile([P, 1024], f32, tag="big")
                        for nc2 in range(2):
                            for kt in range(KT):
                                nc.tensor.matmul(
                                    ps[:, nc2 * 512:(nc2 + 1) * 512],
                                    h1T[:, kt, tt * P:(tt + 1) * P],
                                    wv_sb[:, kt, nc2 * 512:(nc2 + 1) * 512],
                                    start=(kt == 0), stop=(kt == KT - 1))
                        nc.vector.tensor_copy(
                            v_sb[:, tt, :, 0:DH],
                            ps[:].rearrange("p (hh r) -> p hh r", r=DH))

                # Q projection (own half, softmax scale pre-folded into wq1)
                qT = qTp.tile([P, KT, N_OWN], bf16, tag="qT")
                w_sb = wbig.tile([P, KT, INNER], bf16, tag="w")
                nc.sync.dma_start(w_sb[:], wq1_d[:])
                for m in range(KT):
                    ps = ps_big.tile([P, 1024], f32, tag="big")
                    for qc in range(2):
                        for kt in range(KT):
                            nc.tensor.matmul(
                                ps[:, qc * 512:(qc + 1) * 512],
                                w_sb[:, kt, m * P:(m + 1) * P],
                                h1T[:, kt, qc * 512:(qc + 1) * 512],
                                start=(kt == 0), stop=(kt == KT - 1))
                    nc.vector.tensor_copy(qT[:, m, :], ps[:])

                # K projection (full sample) -> DRAM
                w_sb = wbig.tile([P, KT, INNER], bf16, tag="w")
                nc.sync.dma_start(w_sb[:], wk1_d[:])
                for m in range(KT):
                    for half in range(2):
                        ps = ps_big.tile([P, 1024], f32, tag="big")
                        for qc in range(2):
                            for kt in range(KT):
                                nc.tensor.matmul(
                                    ps[:, qc * 512:(qc + 1) * 512],
                                    w_sb[:, kt, m * P:(m + 1) * P],
                                    h1T[:, kt, half * 1024 + qc * 512:
                                        half * 1024 + (qc + 1) * 512],
                                    start=(kt == 0), stop=(kt == KT - 1))
                        kst = stg.tile([P, 1024], bf16, tag="kst")
                        nc.vector.tensor_copy(kst[:], ps[:])
                        nc.sync.dma_start(
                            kT_dram[m, :, half * 1024:(half + 1) * 1024], kst[:])

            # h1T freed here.

            # ---------------- cross K2/V2 (early, PE filler) ----------------
            k2T = vp.tile([P, KT, J], bf16, tag="k2T")
            w_sb = wbig.tile([P, KT, INNER], bf16, tag="w")
            nc.sync.dma_start(w_sb[:, 0:CKT, :], wk2_d[:])
            for m in range(KT):
                ps = ps_av.tile([P, 512], f32, tag="av")
                for kt in range(CKT):
                    nc.tensor.matmul(ps[:, 0:J], w_sb[:, kt, m * P:(m + 1) * P],
                                     ctxT_sb[:, kt, :],
                                     start=(kt == 0), stop=(kt == CKT - 1))
                nc.vector.tensor_copy(k2T[:, m, :], ps[:, 0:J])

            v2_sb = vp.tile([P, J // P, H, DH + 1], bf16, tag="v2")
            nc.vector.memset(v2_sb[:], 1.0)
            w_sb = wbig.tile([P, KT, INNER], bf16, tag="w")
            nc.sync.dma_start(w_sb[:, 0:CKT, :], wv2_d[:])
            for tt in range(J // P):
                ps = ps_big.tile([P, 1024], f32, tag="big")
                for nc2 in range(2):
                    for kt in range(CKT):
                        nc.tensor.matmul(
                            ps[:, nc2 * 512:(nc2 + 1) * 512],
                            ctxT_sb[:, kt, tt * P:(tt + 1) * P],
                            w_sb[:, kt, nc2 * 512:(nc2 + 1) * 512],
                            start=(kt == 0), stop=(kt == CKT - 1))
                nc.vector.tensor_copy(
                    v2_sb[:, tt, :, 0:DH],
                    ps[:].rearrange("p (hh r) -> p hh r", r=DH))

            # ---------------- attention core ----------------
            with tc.tile_pool(name="expp", bufs=2) as expp, \
                 tc.tile_pool(name="atp", bufs=1) as atp, \
                 tc.tile_pool(name="kcp", bufs=2) as kcp:

                def attn_head(h, qc, get_k, v_t, qT_t, nkt, out_T):
                    """One (head, query-chunk) of attention -> out_T slice."""
                    hp = (h % 2) * 64
                    m2 = h // 2
                    qs = slice(qc * 512, (qc + 1) * 512)
                    kap = get_k(h)
                    # scores + exp, 2 key-tiles per PSUM
                    exs = []
                    for half in range((nkt + 7) // 8):
                        ex = expp.tile([P, 8, 512], bf16, tag="ex")
                        exs.append(ex)
                    for kt2 in range((nkt + 1) // 2):
                        ps = ps_big.tile([P, 1024], f32, tag="big")
                        for u in range(min(2, nkt)):
                            kt = kt2 * 2 + u
                            nc.tensor.matmul(
                                ps[:, u * 512:(u + 1) * 512],
                                kap[hp:hp + 64, kt * P:(kt + 1) * P],
                                qT_t[hp:hp + 64, m2, qs],
                                start=True, stop=True)
                        nkk = min(2, nkt)
                        ex = exs[kt2 // 4]
                        lo = (kt2 % 4) * 2
                        nc.scalar.activation(
                            ex[:, lo:lo + nkk, :].rearrange("p a b -> p (a b)"),
                            ps[:, 0:nkk * 512], AF.Exp)
                    # AV with denominator from the ones column of v:
                    # pav[0:64] = data, pav[64] = softmax denominator
                    pav = ps_av.tile([P, 512], f32, tag="av")
                    for kt in range(nkt):
                        nc.tensor.matmul(
                            pav[0:65], v_t[:, kt, h, :],
                            exs[kt // 8][:, kt % 8, :],
                            start=(kt == 0), stop=(kt == nkt - 1))
                    den = small.tile([1, 512], f32, tag="den")
                    nc.vector.tensor_copy(den[:], pav[64:65, :])
                    rec32 = small.tile([1, 512], f32, tag="rec32")
                    nc.vector.reciprocal_approx_fast(rec32[:], den[:])
                    bcs = small.tile([64, 512], f32, tag="bcs")
                    nc.gpsimd.partition_broadcast(bcs[:], rec32[:])
                    if hp == 0:
                        nc.vector.tensor_tensor(out_T[0:64, m2, qs],
                                                pav[0:64], bcs[:], ALU.mult)
                    else:
                        tmp = small.tile([64, 512], bf16, tag="todd")
                        nc.vector.tensor_tensor(tmp[:], pav[0:64],
                                                bcs[:], ALU.mult)
                        nc.sync.dma_start(out_T[64:128, m2, qs], tmp[:])

                def out_proj(attn_T, w_t, brow_i, resid_src, out_dram, tt,
                             ln_grp, h_dst):
                    """o-proj + bias + residual for token tile tt; LN into
                    h_dst."""
                    ps = ps_big.tile([P, 1024], f32, tag="big")
                    for dc in range(2):
                        dsl = slice(dc * 512, (dc + 1) * 512)
                        for m in range(KT):
                            nc.tensor.matmul(ps[:, dsl],
                                             attn_T[:, m, tt * P:(tt + 1) * P],
                                             w_t[:, m, dsl],
                                             start=(m == 0), stop=False)
                        nc.tensor.matmul(
                            ps[:, dsl], ones[0:1, :],
                            brow_sb[0:1, brow_i * D + dc * 512:
                                    brow_i * D + (dc + 1) * 512],
                            start=False, stop=True)
                    rt = xpool.tile([P, D], f32, tag="x")
                    nc.sync.dma_start(rt[:], resid_src[tt])
                    xot = stg.tile([P, D], f32, tag="ostage")
                    nc.vector.tensor_tensor(xot[:], ps[:], rt[:], ALU.add)
                    nc.sync.dma_start(out_dram[tt], xot[:])
                    layernorm_tile(xot, ln_grp, h_dst, tt * P)

                # -------- self-attention + o1 + LN2, qc-interleaved --------
                attn1T = atp.tile([P, KT, N_OWN], bf16, tag="attnT")
                h2T = hTp.tile([P, KT, N_OWN], bf16, tag="hT")
                wo1_sb = wbig.tile([P, KT, INNER], bf16, tag="w")
                nc.sync.dma_start(wo1_sb[:], wo1_d[:])

                _kc = {}

                def get_k_self(h):
                    m2 = h // 2
                    if _kc.get("m2") != m2:
                        kth = kcp.tile([P, N], bf16, tag="kth")
                        nc.sync.dma_start(kth[:], kT_dram[m2])
                        _kc["m2"] = m2
                        _kc["t"] = kth
                    return _kc["t"]

                for qc in range(2):
                    _kc.clear()
                    for h in range(H):
                        attn_head(h, qc, get_k_self, v_sb, qT, TT_FULL, attn1T)
                    for tt in range(qc * 4, qc * 4 + 4):
                        out_proj(attn1T, wo1_sb, 0, xf_d, x1_dram, tt, 1, h2T)
                    if qc == 0:
                        # prefetch q2 weights mid-attention (ring slot of wv2)
                        wq2_sb = wbig.tile([P, KT, INNER], bf16, tag="w")
                        nc.sync.dma_start(wq2_sb[:], wq2_d[:])

                # -------- q2 projection --------
                q2T = qTp.tile([P, KT, N_OWN], bf16, tag="qT")
                for m in range(KT):
                    ps = ps_big.tile([P, 1024], f32, tag="big")
                    for qc in range(2):
                        for kt in range(KT):
                            nc.tensor.matmul(
                                ps[:, qc * 512:(qc + 1) * 512],
                                wq2_sb[:, kt, m * P:(m + 1) * P],
                                h2T[:, kt, qc * 512:(qc + 1) * 512],
                                start=(kt == 0), stop=(kt == KT - 1))
                    nc.vector.tensor_copy(q2T[:, m, :], ps[:])

                # -------- cross-attention + o2 + LN3 --------
                attn2T = atp.tile([P, KT, N_OWN], bf16, tag="attnT")
                h3T = hTp.tile([P, KT, N_OWN], bf16, tag="hT")
                wo2_sb = wbig.tile([P, KT, INNER], bf16, tag="w")
                nc.sync.dma_start(wo2_sb[:], wo2_d[:])

                def get_k_cross(h):
                    return k2T[:, h // 2, :]

                for qc in range(2):
                    for h in range(H):
                        attn_head(h, qc, get_k_cross, v2_sb, q2T, J // P, attn2T)
                    for tt in range(qc * 4, qc * 4 + 4):
                        out_proj(attn2T, wo2_sb, 1, x1_dram, x2_dram, tt, 2, h3T)

        # ---------------- Phase 6: GEGLU FF ----------------
        with tc.tile_pool(name="gp", bufs=1) as gp, \
             tc.tile_pool(name="wf2p", bufs=2) as wf2p:
            g_sb = gp.tile([P, 32, N_OWN], bf16, tag="g")
            for fc in range(32):
                wa = wsm.tile([P, KT, P], bf16, tag="wstream")
                nc.sync.dma_start(wa[:], wf1_d[:, :, fc * P:(fc + 1) * P])
                wg = wsm.tile([P, KT, P], bf16, tag="wstream")
                nc.sync.dma_start(wg[:], wf1_d[:, :, FF + fc * P:FF + (fc + 1) * P])
                for qc in range(2):
                    qs = slice(qc * 512, (qc + 1) * 512)
                    ps = ps_big.tile([P, 1024], f32, tag="big")
                    for kt in range(KT):
                        nc.tensor.matmul(ps[:, 0:512], wa[:, kt, :], h3T[:, kt, qs],
                                         start=(kt == 0), stop=(kt == KT - 1))
                    for kt in range(KT):
                        nc.tensor.matmul(ps[:, 512:1024], wg[:, kt, :],
                                         h3T[:, kt, qs],
                                         start=(kt == 0), stop=(kt == KT - 1))
                    gt = small.tile([P, 512], bf16, tag="gt")
                    nc.scalar.activation(gt[:], ps[:, 512:1024], AF.Gelu,
                                         bias=fb1_sb[:, 32 + fc:32 + fc + 1])
                    nc.vector.scalar_tensor_tensor(
                        g_sb[:, fc, qs], ps[:, 0:512], fb1_sb[:, fc:fc + 1],
                        gt[:], ALU.add, ALU.mult)

            for dc in range(4):
                dsl = slice(dc * 256, (dc + 1) * 256)
                wf2t = wf2p.tile([P, 32, 256], bf16, tag="wf2")
                nc.sync.dma_start(wf2t[:], wf2_d[:, :, dsl])
                for tt in range(TT_OWN):
                    ps = ps_av.tile([P, 512], f32, tag="av")
                    for j in range(32):
                        nc.tensor.matmul(ps[:, 0:256],
                                         g_sb[:, j, tt * P:(tt + 1) * P],
                                         wf2t[:, j, :],
                                         start=(j == 0), stop=False)
                    nc.tensor.matmul(ps[:, 0:256], ones[0:1, :],
                                     brow_sb[0:1, 2 * D + dc * 256:
                                             2 * D + (dc + 1) * 256],
                                     start=False, stop=True)
                    rt = xpool.tile([P, D], f32, tag="x")
                    nc.sync.dma_start(rt[:, 0:256], x2_dram[tt, :, dsl])
                    yt = stg.tile([P, D], f32, tag="ostage")
                    nc.vector.tensor_tensor(yt[:, 0:256], ps[:, 0:256],
                                            rt[:, 0:256], ALU.add)
                    nc.sync.dma_start(y_d[tt, :, dsl], yt[:, 0:256])

    nc.compile()
    return nc


def _rearr_w(w, kt):
    return np.ascontiguousarray(
        w.reshape(kt, P, -1).transpose(1, 0, 2)).astype(BF16)


def _shard_inputs(inputs):
    f = {k: np.asarray(v, dtype=np.float32) for k, v in inputs.items()}
    shared = {
        "nw": _rearr_w(np.concatenate([f["n1_w"], f["n2_w"], f["n3_w"]], axis=1), KT),
        "nbc": np.ascontiguousarray(
            np.concatenate([f["n1_b"], f["n2_b"], f["n3_b"]])
            .reshape(3, 16, P).transpose(2, 0, 1).reshape(P, 48)),
        "wq1": _rearr_w(f["q1"] * SC, KT), "wk1": _rearr_w(f["k1"], KT),
        "wv1": _rearr_w(f["v1"], KT), "wo1": _rearr_w(f["o1_w"], KT),
        "wq2": _rearr_w(f["q2"] * SC, KT), "wk2": _rearr_w(f["k2"], CKT),
        "wv2": _rearr_w(f["v2"], CKT), "wo2": _rearr_w(f["o2_w"], KT),
        "brow": np.ascontiguousarray(
            np.concatenate([f["o1_b"], f["o2_b"], f["ff_b2"]])
            .reshape(1, 3 * D)).astype(BF16),
        "fb1c": np.ascontiguousarray(f["ff_b1"].reshape(64, P).T),
        "wf1": _rearr_w(f["ff_w1"], KT),
        "wf2": _rearr_w(f["ff_w2"], FF // P),
    }
    in_maps = []
    for core in range(8):
        b, half = core // 2, core % 2
        own = f["x"][b, half * N_OWN:(half + 1) * N_OWN]
        oth = f["x"][b, (1 - half) * N_OWN:(2 - half) * N_OWN]
        m = dict(shared)
        m["xf"] = np.ascontiguousarray(
            np.concatenate([own, oth]).reshape(TT_FULL, P, D))
        m["tT"] = np.ascontiguousarray(f["t"][b, 0].reshape(KT, P).T).astype(BF16)
        m["ctxT"] = np.ascontiguousarray(
            f["context"][b].T.reshape(CKT, P, J).transpose(1, 0, 2)).astype(BF16)
        in_maps.append(m)
    return in_maps


def kernel(**inputs):
    from concourse.bass_utils import run_bass_kernel_spmd
    if "nc" not in _CACHE:
        _CACHE["nc"] = _build_program()
    nc = _CACHE["nc"]
    in_maps = _shard_inputs(inputs)
    res = run_bass_kernel_spmd(nc, in_maps, core_ids=list(range(8)))
    out = np.empty((B, N, D), dtype=np.float32)
    for core in range(8):
        b, half = core // 2, core % 2
        out[b, half * N_OWN:(half + 1) * N_OWN] = \
            res.results[core]["y"].reshape(N_OWN, D)
    return out


# revision 13
# speedup vs baseline: 1.8671x; 1.0134x over previous
"""Trainium2 Bass kernel for nn_BasicTransformerBlock_35304631173827.

Sharding: 8 cores = 4 samples x 2 sequence halves. Each core computes its
1024-token half of one sample fully locally (self-attention K/V recomputed
over the full 2048-token sample -> zero collectives). bf16 matmuls with
fp32 PSUM accumulation; LayerNorm stats, softmax and residuals in fp32.

v2 restructure vs baseline:
- h1T kept SBUF-resident; dense projections use paired 512-col PSUM chains.
- Attention: every head's AV matmul is M=65 with a ones-column in V so the
  softmax denominator falls out of the same accumulation chain (no separate
  M=1 denominator matmuls); reciprocal_approx_fast for 1/den; odd heads'
  outputs shifted to partitions 64:128 via a small SBUF->SBUF DMA.
- qc-outer / head-inner loop with o-proj + LN interleaved to keep the PE
  array busy (p-state ramp) while the scalar engine chews softmax exps.
- FF: PSUM-accumulated FF2 (full K=4096 contraction in one chain), fused
  (a+b1)*gelu(gate+b1') via scalar_tensor_tensor, output biases folded into
  the matmul chains as K=1 ones-row accumulation steps.
"""

import numpy as np
import ml_dtypes

BF16 = ml_dtypes.bfloat16

B, N, D = 4, 2048, 1024
J, CD = 256, 768
H, DH = 16, 64
INNER = 1024
FF = 4096
P = 128
KT = D // P            # 8
CKT = CD // P          # 6
TT_FULL = N // P       # 16
N_OWN = N // 2
TT_OWN = N_OWN // P    # 8
EPS = 1e-5
SC = DH ** -0.5

_CACHE = {}


def _build_program():
    import concourse.tile as tile
    from concourse import mybir, bacc
    from concourse.masks import make_identity
    from contextlib import ExitStack

    f32 = mybir.dt.float32
    bf16 = mybir.dt.bfloat16
    AF = mybir.ActivationFunctionType
    ALU = mybir.AluOpType

    nc = bacc.Bacc(None, target_bir_lowering=False)

    xf_d = nc.dram_tensor("xf", [TT_FULL, P, D], f32, kind="ExternalInput")
    tT_d = nc.dram_tensor("tT", [P, KT], bf16, kind="ExternalInput")
    nw_d = nc.dram_tensor("nw", [P, KT, 6 * D], bf16, kind="ExternalInput")
    nbc_d = nc.dram_tensor("nbc", [P, 48], f32, kind="ExternalInput")
    wq1_d = nc.dram_tensor("wq1", [P, KT, INNER], bf16, kind="ExternalInput")
    wk1_d = nc.dram_tensor("wk1", [P, KT, INNER], bf16, kind="ExternalInput")
    wv1_d = nc.dram_tensor("wv1", [P, KT, INNER], bf16, kind="ExternalInput")
    wo1_d = nc.dram_tensor("wo1", [P, KT, D], bf16, kind="ExternalInput")
    wq2_d = nc.dram_tensor("wq2", [P, KT, INNER], bf16, kind="ExternalInput")
    wk2_d = nc.dram_tensor("wk2", [P, CKT, INNER], bf16, kind="ExternalInput")
    wv2_d = nc.dram_tensor("wv2", [P, CKT, INNER], bf16, kind="ExternalInput")
    wo2_d = nc.dram_tensor("wo2", [P, KT, D], bf16, kind="ExternalInput")
    ctxT_d = nc.dram_tensor("ctxT", [P, CKT, J], bf16, kind="ExternalInput")
    brow_d = nc.dram_tensor("brow", [1, 3 * D], bf16, kind="ExternalInput")
    fb1_d = nc.dram_tensor("fb1c", [P, 64], f32, kind="ExternalInput")
    wf1_d = nc.dram_tensor("wf1", [P, KT, 2 * FF], bf16, kind="ExternalInput")
    wf2_d = nc.dram_tensor("wf2", [P, FF // P, D], bf16, kind="ExternalInput")
    y_d = nc.dram_tensor("y", [TT_OWN, P, D], f32, kind="ExternalOutput")

    # DRAM scratch
    kT_dram = nc.dram_tensor("scr_kT", [KT, P, N], bf16, kind="Internal")
    x1_dram = nc.dram_tensor("scr_x1", [TT_OWN, P, D], f32, kind="Internal")
    x2_dram = nc.dram_tensor("scr_x2", [TT_OWN, P, D], f32, kind="Internal")

    with tile.TileContext(nc) as tc, ExitStack() as es:
        konst = es.enter_context(tc.tile_pool(name="konst", bufs=1))
        xpool = es.enter_context(tc.tile_pool(name="xpool", bufs=2))
        stats = es.enter_context(tc.tile_pool(name="stats", bufs=3))
        small = es.enter_context(tc.tile_pool(name="small", bufs=2))
        wsm = es.enter_context(tc.tile_pool(name="wsm", bufs=4))
        wbig = es.enter_context(tc.tile_pool(name="wbig", bufs=2))
        stg = es.enter_context(tc.tile_pool(name="stg", bufs=2))
        hTp = es.enter_context(tc.tile_pool(name="hTp", bufs=1))
        ps_big = es.enter_context(tc.tile_pool(name="ps_big", bufs=2, space="PSUM"))
        ps_av = es.enter_context(tc.tile_pool(name="ps_av", bufs=2, space="PSUM"))
        ps_bc = es.enter_context(tc.tile_pool(name="ps_bc", bufs=2, space="PSUM"))

        # ---------------- constants ----------------
        ident = konst.tile([P, P], bf16)
        make_identity(nc, ident)
        ones = konst.tile([1, P], bf16)
        nc.vector.memset(ones[:], 1.0)
        eps_t = konst.tile([P, 1], f32)
        nc.vector.memset(eps_t[:], EPS)
        tT_sb = konst.tile([P, KT], bf16)
        nc.sync.dma_start(tT_sb[:], tT_d[:])
        nbc_sb = konst.tile([P, 48], f32)
        nc.sync.dma_start(nbc_sb[:], nbc_d[:])
        fb1_sb = konst.tile([P, 64], f32)
        nc.sync.dma_start(fb1_sb[:], fb1_d[:])
        ctxT_sb = konst.tile([P, CKT, J], bf16)
        nc.sync.dma_start(ctxT_sb[:], ctxT_d[:])
        brow_sb = konst.tile([1, 3 * D], bf16)
        nc.sync.dma_start(brow_sb[:], brow_d[:])
        cols = konst.tile([P, 48], f32)

        # ---------------- Phase 0: AdaLN embeddings ----------------
        # emb^T chunks: cols[:, cc] = (t @ nW)[cc*128 : (cc+1)*128]
        for g in range(3):
            for c in range(16):
                cc = g * 16 + c
                nwt = wsm.tile([P, KT, P], bf16, tag="wstream")
                nc.sync.dma_start(nwt[:], nw_d[:, :, cc * P:(cc + 1) * P])
                ps = ps_av.tile([P, 512], f32, tag="av")
                for kt in range(KT):
                    nc.tensor.matmul(ps[:, 0:1], nwt[:, kt, :], tT_sb[:, kt:kt + 1],
                                     start=(kt == 0), stop=(kt == KT - 1))
                nc.vector.tensor_copy(cols[:, cc:cc + 1], ps[:, 0:1])
            sl = slice(g * 16, g * 16 + 16)
            nc.vector.tensor_add(cols[:, sl], cols[:, sl], nbc_sb[:, sl])
            nc.vector.tensor_scalar_add(cols[:, g * 16:g * 16 + 8],
                                        cols[:, g * 16:g * 16 + 8], 1.0)

        def layernorm_tile(x_tile, n3, dst_sb, off):
            """LayerNorm + AdaLN affine on (P, D) tile -> transposed chunks
            written to dst_sb[:, c, off:off+128]."""
            bst = stats.tile([P, 2, 6], f32, tag="bnst")
            for g in range(2):
                nc.vector.bn_stats(bst[:, g, :], x_tile[:, g * 512:(g + 1) * 512])
            mv = stats.tile([P, 4], f32, tag="mv")
            nc.vector.bn_aggr(mv[:, 0:2], bst[:])
            nc.scalar.activation(mv[:, 2:3], mv[:, 1:2], AF.Sqrt, bias=eps_t[:])
            nc.vector.reciprocal(mv[:, 2:3], mv[:, 2:3])
            nc.vector.tensor_tensor(mv[:, 3:4], mv[:, 0:1], mv[:, 2:3], ALU.mult)
            nc.vector.tensor_scalar_mul(mv[:, 3:4], mv[:, 3:4], -1.0)
            xn = stats.tile([P, D], bf16, tag="xn")
            nc.scalar.activation(xn[:], x_tile[:], AF.Identity,
                                 bias=mv[:, 3:4], scale=mv[:, 2:3])
            for c in range(KT):
                pt = ps_bc.tile([P, P], bf16, tag="bc")
                nc.tensor.transpose(pt[:], xn[:, c * P:(c + 1) * P], ident[:])
                nc.vector.tensor_scalar(
                    dst_sb[:, c, off:off + P], pt[:],
                    cols[:, n3 * 16 + c:n3 * 16 + c + 1],
                    cols[:, n3 * 16 + 8 + c:n3 * 16 + 8 + c + 1],
                    ALU.mult, ALU.add)

        # outer-attention scope: qT/q2T, v tiles
        with tc.tile_pool(name="qTp", bufs=1) as qTp, \
             tc.tile_pool(name="vp", bufs=1) as vp:

            # ---------------- Phase 1+2: LN1 -> h1T (SBUF); QKV ----------------
            # LN1 interleaved with V-projection in 4-tile token groups so the
            # PE array has matmul work while LN chains stream through DVE.
            with tc.tile_pool(name="h1p", bufs=1) as h1p:
                h1T = h1p.tile([P, KT, N], bf16, tag="h1T")
                v_sb = vp.tile([P, TT_FULL, H, DH + 1], bf16, tag="v1")
                nc.vector.memset(v_sb[:], 1.0)
                wv_sb = wbig.tile([P, KT, INNER], bf16, tag="w")
                nc.sync.dma_start(wv_sb[:], wv1_d[:])
                def v_group(grp):
                    for tt in range(grp * 4, grp * 4 + 4):
                        ps = ps_big.tile([P, 1024], f32, tag="big")
                        for nc2 in range(2):
                            for kt in range(KT):
                                nc.tensor.matmul(
                                    ps[:, nc2 * 512:(nc2 + 1) * 512],
                                    h1T[:, kt, tt * P:(tt + 1) * P],
                                    wv_sb[:, kt, nc2 * 512:(nc2 + 1) * 512],
                                    start=(kt == 0), stop=(kt == KT - 1))
                        nc.vector.tensor_copy(
                            v_sb[:, tt, :, 0:DH],
                            ps[:].rearrange("p (hh r) -> p hh r", r=DH))

                for grp in range(4):
                    for tt in range(grp * 4, grp * 4 + 4):
                        xt = xpool.tile([P, D], f32, tag="x")
                        nc.sync.dma_start(xt[:], xf_d[tt])
                        layernorm_tile(xt, 0, h1T, tt * P)
                    if grp > 0:
                        v_group(grp - 1)
                v_group(3)

                # K projection (full sample) -> DRAM
                w_sb = wbig.tile([P, KT, INNER], bf16, tag="w")
                nc.sync.dma_start(w_sb[:], wk1_d[:])
                for m in range(KT):
                    for half in range(2):
                        ps = ps_big.tile([P, 1024], f32, tag="big")
                        for qc in range(2):
                            for kt in range(KT):
                                nc.tensor.matmul(
                                    ps[:, qc * 512:(qc + 1) * 512],
                                    w_sb[:, kt, m * P:(m + 1) * P],
                                    h1T[:, kt, half * 1024 + qc * 512:
                                        half * 1024 + (qc + 1) * 512],
                                    start=(kt == 0), stop=(kt == KT - 1))
                        kst = stg.tile([P, 1024], bf16, tag="kst")
                        nc.vector.tensor_copy(kst[:], ps[:])
                        nc.sync.dma_start(
                            kT_dram[m, :, half * 1024:(half + 1) * 1024], kst[:])

                # Q projection (own half, softmax scale pre-folded into wq1)
                qT = qTp.tile([P, KT, N_OWN], bf16, tag="qT")
                w_sb = wbig.tile([P, KT, INNER], bf16, tag="w")
                nc.sync.dma_start(w_sb[:], wq1_d[:])
                for m in range(KT):
                    ps = ps_big.tile([P, 1024], f32, tag="big")
                    for qc in range(2):
                        for kt in range(KT):
                            nc.tensor.matmul(
                                ps[:, qc * 512:(qc + 1) * 512],
                                w_sb[:, kt, m * P:(m + 1) * P],
                                h1T[:, kt, qc * 512:(qc + 1) * 512],
                                start=(kt == 0), stop=(kt == KT - 1))
                    nc.vector.tensor_copy(qT[:, m, :], ps[:])

            # h1T freed here.

            # ---------------- cross K2/V2 (early, PE filler) ----------------
            k2T = vp.tile([P, KT, J], bf16, tag="k2T")
            w_sb = wbig.tile([P, KT, INNER], bf16, tag="w")
            nc.sync.dma_start(w_sb[:, 0:CKT, :], wk2_d[:])
            for m in range(KT):
                ps = ps_av.tile([P, 512], f32, tag="av")
                for kt in range(CKT):
                    nc.tensor.matmul(ps[:, 0:J], w_sb[:, kt, m * P:(m + 1) * P],
                                     ctxT_sb[:, kt, :],
                                     start=(kt == 0), stop=(kt == CKT - 1))
                nc.vector.tensor_copy(k2T[:, m, :], ps[:, 0:J])

            v2_sb = vp.tile([P, J // P, H, DH + 1], bf16, tag="v2")
            nc.vector.memset(v2_sb[:], 1.0)
            w_sb = wbig.tile([P, KT, INNER], bf16, tag="w")
            nc.sync.dma_start(w_sb[:, 0:CKT, :], wv2_d[:])
            for tt in range(J // P):
                ps = ps_big.tile([P, 1024], f32, tag="big")
                for nc2 in range(2):
                    for kt in range(CKT):
                        nc.tensor.matmul(
                            ps[:, nc2 * 512:(nc2 + 1) * 512],
                            ctxT_sb[:, kt, tt * P:(tt + 1) * P],
                            w_sb[:, kt, nc2 * 512:(nc2 + 1) * 512],
                            start=(kt == 0), stop=(kt == CKT - 1))
                nc.vector.tensor_copy(
                    v2_sb[:, tt, :, 0:DH],
                    ps[:].rearrange("p (hh r) -> p hh r", r=DH))

            # ---------------- attention core ----------------
            with tc.tile_pool(name="expp", bufs=2) as expp, \
                 tc.tile_pool(name="atp", bufs=1) as atp, \
                 tc.tile_pool(name="kcp", bufs=2) as kcp:

                def attn_head(h, qc, get_k, v_t, qT_t, nkt, out_T,
                              den_scalar=False):
                    """One (head, query-chunk) of attention -> out_T slice."""
                    hp = (h % 2) * 64
                    m2 = h // 2
                    qs = slice(qc * 512, (qc + 1) * 512)
                    kap = get_k(h)
                    # scores + exp, 2 key-tiles per PSUM
                    exs = []
                    for half in range((nkt + 7) // 8):
                        ex = expp.tile([P, 8, 512], bf16, tag="ex")
                        exs.append(ex)
                    for kt2 in range((nkt + 1) // 2):
                        ps = ps_big.tile([P, 1024], f32, tag="big")
                        for u in range(min(2, nkt)):
                            kt = kt2 * 2 + u
                            nc.tensor.matmul(
                                ps[:, u * 512:(u + 1) * 512],
                                kap[hp:hp + 64, kt * P:(kt + 1) * P],
                                qT_t[hp:hp + 64, m2, qs],
                                start=True, stop=True)
                        nkk = min(2, nkt)
                        ex = exs[kt2 // 4]
                        lo = (kt2 % 4) * 2
                        nc.scalar.activation(
                            ex[:, lo:lo + nkk, :].rearrange("p a b -> p (a b)"),
                            ps[:, 0:nkk * 512], AF.Exp)
                    # AV with denominator from the ones column of v:
                    # pav[0:64] = data, pav[64] = softmax denominator
                    pav = ps_av.tile([P, 512], f32, tag="av")
                    for kt in range(nkt):
                        nc.tensor.matmul(
                            pav[0:65], v_t[:, kt, h, :],
                            exs[kt // 8][:, kt % 8, :],
                            start=(kt == 0), stop=(kt == nkt - 1))
                    den = small.tile([1, 512], f32, tag="den")
                    if den_scalar:
                        nc.scalar.activation(den[:], pav[64:65, :], AF.Copy)
                    else:
                        nc.vector.tensor_copy(den[:], pav[64:65, :])
                    rec32 = small.tile([1, 512], f32, tag="rec32")
                    nc.vector.reciprocal_approx_fast(rec32[:], den[:])
                    bcs = small.tile([64, 512], f32, tag="bcs")
                    nc.gpsimd.partition_broadcast(bcs[:], rec32[:])
                    if hp == 0:
                        nc.vector.tensor_tensor(out_T[0:64, m2, qs],
                                                pav[0:64], bcs[:], ALU.mult)
                    else:
                        tmp = small.tile([64, 512], bf16, tag="todd")
                        nc.vector.tensor_tensor(tmp[:], pav[0:64],
                                                bcs[:], ALU.mult)
                        nc.sync.dma_start(out_T[64:128, m2, qs], tmp[:])

                def out_proj(attn_T, w_t, brow_i, resid_src, out_dram, tt,
                             ln_grp, h_dst):
                    """o-proj + bias + residual for token tile tt; LN into
                    h_dst."""
                    ps = ps_big.tile([P, 1024], f32, tag="big")
                    for dc in range(2):
                        dsl = slice(dc * 512, (dc + 1) * 512)
                        for m in range(KT):
                            nc.tensor.matmul(ps[:, dsl],
                                             attn_T[:, m, tt * P:(tt + 1) * P],
                                             w_t[:, m, dsl],
                                             start=(m == 0), stop=False)
                        nc.tensor.matmul(
                            ps[:, dsl], ones[0:1, :],
                            brow_sb[0:1, brow_i * D + dc * 512:
                                    brow_i * D + (dc + 1) * 512],
                            start=False, stop=True)
                    rt = xpool.tile([P, D], f32, tag="x")
                    nc.sync.dma_start(rt[:], resid_src[tt])
                    xot = stg.tile([P, D], f32, tag="ostage")
                    nc.vector.tensor_tensor(xot[:], ps[:], rt[:], ALU.add)
                    nc.sync.dma_start(out_dram[tt], xot[:])
                    layernorm_tile(xot, ln_grp, h_dst, tt * P)

                # -------- self-attention + o1 + LN2, qc-interleaved --------
                attn1T = atp.tile([P, KT, N_OWN], bf16, tag="attnT")
                h2T = hTp.tile([P, KT, N_OWN], bf16, tag="hT")
                wo1_sb = wbig.tile([P, KT, INNER], bf16, tag="w")
                nc.sync.dma_start(wo1_sb[:], wo1_d[:])

                _kc = {}

                def get_k_self(h):
                    m2 = h // 2
                    if _kc.get("m2") != m2:
                        kth = kcp.tile([P, N], bf16, tag="kth")
                        nc.sync.dma_start(kth[:], kT_dram[m2])
                        _kc["m2"] = m2
                        _kc["t"] = kth
                    return _kc["t"]

                for qc in range(2):
                    _kc.clear()
                    for h in range(H):
                        attn_head(h, qc, get_k_self, v_sb, qT, TT_FULL, attn1T)
                    for tt in range(qc * 4, qc * 4 + 4):
                        out_proj(attn1T, wo1_sb, 0, xf_d, x1_dram, tt, 1, h2T)
                    if qc == 0:
                        # prefetch q2 weights mid-attention (ring slot of wv2)
                        wq2_sb = wbig.tile([P, KT, INNER], bf16, tag="w")
                        nc.sync.dma_start(wq2_sb[:], wq2_d[:])

                # -------- q2 projection --------
                q2T = qTp.tile([P, KT, N_OWN], bf16, tag="qT")
                for m in range(KT):
                    ps = ps_big.tile([P, 1024], f32, tag="big")
                    for qc in range(2):
                        for kt in range(KT):
                            nc.tensor.matmul(
                                ps[:, qc * 512:(qc + 1) * 512],
                                wq2_sb[:, kt, m * P:(m + 1) * P],
                                h2T[:, kt, qc * 512:(qc + 1) * 512],
                                start=(kt == 0), stop=(kt == KT - 1))
                    nc.vector.tensor_copy(q2T[:, m, :], ps[:])

                # -------- cross-attention + o2 + LN3 --------
                attn2T = atp.tile([P, KT, N_OWN], bf16, tag="attnT")
                h3T = hTp.tile([P, KT, N_OWN], bf16, tag="hT")
                wo2_sb = wbig.tile([P, KT, INNER], bf16, tag="w")
                nc.sync.dma_start(wo2_sb[:], wo2_d[:])

                def get_k_cross(h):
                    return k2T[:, h // 2, :]

                for qc in range(2):
                    for h in range(H):
                        attn_head(h, qc, get_k_cross, v2_sb, q2T, J // P,
                                  attn2T, den_scalar=True)
                    for tt in range(qc * 4, qc * 4 + 4):
                        out_proj(attn2T, wo2_sb, 1, x1_dram, x2_dram, tt, 2, h3T)

        # ---------------- Phase 6: GEGLU FF ----------------
        with tc.tile_pool(name="gp", bufs=1) as gp, \
             tc.tile_pool(name="wf2p", bufs=2) as wf2p:
            g_sb = gp.tile([P, 32, N_OWN], bf16, tag="g")
            for fc in range(32):
                wa = wsm.tile([P, KT, P], bf16, tag="wstream")
                nc.sync.dma_start(wa[:], wf1_d[:, :, fc * P:(fc + 1) * P])
                wg = wsm.tile([P, KT, P], bf16, tag="wstream")
                nc.sync.dma_start(wg[:], wf1_d[:, :, FF + fc * P:FF + (fc + 1) * P])
                for qc in range(2):
                    qs = slice(qc * 512, (qc + 1) * 512)
                    ps = ps_big.tile([P, 1024], f32, tag="big")
                    for kt in range(KT):
                        nc.tensor.matmul(ps[:, 0:512], wa[:, kt, :], h3T[:, kt, qs],
                                         start=(kt == 0), stop=(kt == KT - 1))
                    for kt in range(KT):
                        nc.tensor.matmul(ps[:, 512:1024], wg[:, kt, :],
                                         h3T[:, kt, qs],
                                         start=(kt == 0), stop=(kt == KT - 1))
                    gt = small.tile([P, 512], bf16, tag="gt")
                    nc.scalar.activation(gt[:], ps[:, 512:1024], AF.Gelu,
                                         bias=fb1_sb[:, 32 + fc:32 + fc + 1])
                    nc.vector.scalar_tensor_tensor(
                        g_sb[:, fc, qs], ps[:, 0:512], fb1_sb[:, fc:fc + 1],
                        gt[:], ALU.add, ALU.mult)

            for dc in range(4):
                dsl = slice(dc * 256, (dc + 1) * 256)
                wf2t = wf2p.tile([P, 32, 256], bf16, tag="wf2")
                nc.sync.dma_start(wf2t[:], wf2_d[:, :, dsl])
                for tt in range(TT_OWN):
                    ps = ps_av.tile([P, 512], f32, tag="av")
                    for j in range(32):
                        nc.tensor.matmul(ps[:, 0:256],
                                         g_sb[:, j, tt * P:(tt + 1) * P],
                                         wf2t[:, j, :],
                                         start=(j == 0), stop=False)
                    nc.tensor.matmul(ps[:, 0:256], ones[0:1, :],
                                     brow_sb[0:1, 2 * D + dc * 256:
                                             2 * D + (dc + 1) * 256],
                                     start=False, stop=True)
                    rt = xpool.tile([P, D], f32, tag="x")
                    nc.sync.dma_start(rt[:, 0:256], x2_dram[tt, :, dsl])
                    yt = stg.tile([P, D], f32, tag="ostage")
                    nc.vector.tensor_tensor(yt[:, 0:256], ps[:, 0:256],
                                            rt[:, 0:256], ALU.add)
                    nc.sync.dma_start(y_d[tt, :, dsl], yt[:, 0:256])

    nc.compile()
    return nc


def _rearr_w(w, kt):
    return np.ascontiguousarray(
        w.reshape(kt, P, -1).transpose(1, 0, 2)).astype(BF16)


def _shard_inputs(inputs):
    f = {k: np.asarray(v, dtype=np.float32) for k, v in inputs.items()}
    shared = {
        "nw": _rearr_w(np.concatenate([f["n1_w"], f["n2_w"], f["n3_w"]], axis=1), KT),
        "nbc": np.ascontiguousarray(
            np.concatenate([f["n1_b"], f["n2_b"], f["n3_b"]])
            .reshape(3, 16, P).transpose(2, 0, 1).reshape(P, 48)),
        "wq1": _rearr_w(f["q1"] * SC, KT), "wk1": _rearr_w(f["k1"], KT),
        "wv1": _rearr_w(f["v1"], KT), "wo1": _rearr_w(f["o1_w"], KT),
        "wq2": _rearr_w(f["q2"] * SC, KT), "wk2": _rearr_w(f["k2"], CKT),
        "wv2": _rearr_w(f["v2"], CKT), "wo2": _rearr_w(f["o2_w"], KT),
        "brow": np.ascontiguousarray(
            np.concatenate([f["o1_b"], f["o2_b"], f["ff_b2"]])
            .reshape(1, 3 * D)).astype(BF16),
        "fb1c": np.ascontiguousarray(f["ff_b1"].reshape(64, P).T),
        "wf1": _rearr_w(f["ff_w1"], KT),
        "wf2": _rearr_w(f["ff_w2"], FF // P),
    }
    in_maps = []
    for core in range(8):
        b, half = core // 2, core % 2
        own = f["x"][b, half * N_OWN:(half + 1) * N_OWN]
        oth = f["x"][b, (1 - half) * N_OWN:(2 - half) * N_OWN]
        m = dict(shared)
        m["xf"] = np.ascontiguousarray(
            np.concatenate([own, oth]).reshape(TT_FULL, P, D))
        m["tT"] = np.ascontiguousarray(f["t"][b, 0].reshape(KT, P).T).astype(BF16)
        m["ctxT"] = np.ascontiguousarray(
            f["context"][b].T.reshape(CKT, P, J).transpose(1, 0, 2)).astype(BF16)
        in_maps.append(m)
    return in_maps


def kernel(**inputs):
    from concourse.bass_utils import run_bass_kernel_spmd
    if "nc" not in _CACHE:
        _CACHE["nc"] = _build_program()
    nc = _CACHE["nc"]
    in_maps = _shard_inputs(inputs)
    res = run_bass_kernel_spmd(nc, in_maps, core_ids=list(range(8)))
    out = np.empty((B, N, D), dtype=np.float32)
    for core in range(8):
        b, half = core // 2, core % 2
        out[b, half * N_OWN:(half + 1) * N_OWN] = \
            res.results[core]["y"].reshape(N_OWN, D)
    return out
